# revision 1
# baseline (speedup 1.0000x reference)
"""Causal self-attention (B=4, T=2048, D=1024, H=16) on 8 TRN2 NeuronCores.

Sharding: 2D (batch x head-group). Core c handles batch b = c//2 and head
group g = c%2 (8 heads). Within a core, heads are processed in pairs so the
two 64-deep QK^T matmuls of a pair row-tile the 128-deep PE array.

Layout strategy (per core):
  - x is passed pre-transposed from host: xT [D, T].
  - QKV projections produce qT/kT [128 local dims, T] with head pair 2p/2p+1
    stacked on partitions 0-63 / 64-127, pair blocks along the free dim.
  - Scores are computed transposed: S^T [keys, queries] so that softmax'd
    P^T feeds the PV matmul directly as the moving operand.
  - V is transposed on-device (PE transpose) into natural [token, dim]
    layout, augmented with a ones column per head so the PV matmul also
    accumulates the softmax denominator (row 64 of the [65, 512] output).
  - exp() runs without max-subtraction: inputs are unit-normal scaled, so
    scores are ~N(0,1); fp32 exp cannot overflow here.
  - o_proj consumes y^T directly as the stationary operand; each core emits
    a partial [T, D] product over its 512 local head dims; host sums the
    two partials per batch.

All matmuls use float32r (TF32-style) operands: full PE rate at moving
dim >= 256, ~11 mantissa bits.
"""

import os
import sys

import numpy as np

if not any(os.path.isdir(os.path.join(p, "concourse")) for p in sys.path):
    sys.path.insert(0, "/opt/trn_rl_repo")

import concourse.mybir as mybir
import concourse.tile as tile
from concourse import bacc
from concourse.bass_utils import run_bass_kernel_spmd

B, T, D, H, DH = 4, 2048, 1024, 16, 64
N_CORES = 8
GROUPS = 2          # head groups (tensor-parallel dim)
HPG = H // GROUPS   # heads per group/core
PAIRS = HPG // 2    # head pairs per core
NKB = T // 128      # 128-key blocks per batch
NQT = T // 512      # 512-query tiles per batch
VSTRIDE = NKB * 130 # vnat cols per pair: 16 blocks x [64 dims|1|64 dims|1]

F32 = mybir.dt.float32
F32R = mybir.dt.float32r




def build_nc():
    nc = bacc.Bacc("TRN2", target_bir_lowering=False, debug=False,
                   num_devices=N_CORES)
    xT = nc.dram_tensor("xT", [D, T], F32R, kind="ExternalInput").ap()
    wqT = nc.dram_tensor("wqT", [D, 512], F32R, kind="ExternalInput").ap()
    wkT = nc.dram_tensor("wkT", [D, 512], F32R, kind="ExternalInput").ap()
    wvT = nc.dram_tensor("wvT", [D, 512], F32R, kind="ExternalInput").ap()
    woT = nc.dram_tensor("woT", [512, D], F32R, kind="ExternalInput").ap()
    tri = nc.dram_tensor("tri", [128, 128], F32R, kind="ExternalInput").ap()
    ident = nc.dram_tensor("ident", [128, 128], F32R, kind="ExternalInput").ap()
    ones = nc.dram_tensor("ones", [128, 128], F32R, kind="ExternalInput").ap()
    out = nc.dram_tensor("out", [T, D], F32, kind="ExternalOutput").ap()

    with tile.TileContext(nc) as tc:
        _body(tc, out, xT, wqT, wkT, wvT, woT, tri, ident, ones)
    nc.compile()
    return nc


def _body(tc, out, xT, wqT, wkT, wvT, woT, tri, ident, ones):
    nc = tc.nc
    from contextlib import ExitStack

    with ExitStack() as ctx:
        persist = ctx.enter_context(tc.tile_pool(name="persist", bufs=1))
        qT = persist.tile([128, PAIRS * T], F32R, tag="qT")
        kT = persist.tile([128, PAIRS * T], F32R, tag="kT")
        vnat = persist.tile([128, PAIRS * VSTRIDE], F32R, tag="vnat")
        ynorm = persist.tile([128, PAIRS * T], F32R, tag="ynorm")
        consts = ctx.enter_context(tc.tile_pool(name="consts", bufs=1))
        tri_sb = consts.tile([128, 128], F32R, tag="tri")
        nc.sync.dma_start(tri_sb[:], tri[:])
        ident_sb = consts.tile([128, 128], F32R, tag="ident")
        nc.sync.dma_start(ident_sb[:], ident[:])

        # ones columns of vnat (denominator accumulators): cols 64 and 129
        # of each 130-wide [dims|1|dims|1] block, DMA'd from a DRAM constant
        # (memset can't write float32r).
        ones_view = vnat[:].rearrange("r (p k m x) -> r (p k m) x",
                                      p=PAIRS, k=NKB, m=2)[:, :, 64:65]
        nc.sync.dma_start(ones_view.squeeze(), ones[:])

        # ---------------- Phase A: projections -----------------------
        with ExitStack() as actx:
            xpool = actx.enter_context(tc.tile_pool(name="xt", bufs=1))
            wpool = actx.enter_context(tc.tile_pool(name="w", bufs=2))
            pspool = actx.enter_context(
                tc.tile_pool(name="ps", bufs=3, space="PSUM"))
            tpool = actx.enter_context(
                tc.tile_pool(name="tps", bufs=2, space="PSUM"))
            vtpool = actx.enter_context(tc.tile_pool(name="vt", bufs=2))

            for half in range(2):
                x_sb = []
                for c in range(8):
                    xt = xpool.tile([128, 1024], F32R, tag=f"x{c}")
                    nc.sync.dma_start(
                        xt[:], xT[c * 128:(c + 1) * 128,
                                  half * 1024:(half + 1) * 1024])
                    x_sb.append(xt)
                for kind, wap in (("q", wqT), ("k", wkT), ("v", wvT)):
                    for p in range(PAIRS):
                        w_sb = wpool.tile([128, 1024], F32R, tag="w")
                        wsrc = wap[:, p * 128:(p + 1) * 128]
                        nc.sync.dma_start(
                            w_sb[:].rearrange("r (c o) -> r c o", o=128),
                            wsrc.rearrange("(c r) o -> r c o", r=128))
                        for tt in range(2):
                            ps = pspool.tile([128, 512], F32, tag="ps")
                            for c in range(8):
                                nc.tensor.matmul(
                                    ps[:],
                                    lhsT=(w_sb[:, c * 128:(c + 1) * 128]),
                                    rhs=(x_sb[c][:, tt * 512:(tt + 1) * 512]),
                                    start=(c == 0), stop=(c == 7))
                            col0 = p * T + half * 1024 + tt * 512
                            if kind == "q":
                                nc.scalar.mul(qT[:, col0:col0 + 512], ps[:],
                                              1.0 / np.sqrt(DH))
                            elif kind == "k":
                                nc.scalar.copy(kT[:, col0:col0 + 512], ps[:])
                            else:
                                vt = vtpool.tile([128, 512], F32R, tag="vt")
                                nc.vector.tensor_copy(vt[:], ps[:])
                                for s in range(4):
                                    tps = tpool.tile([128, 128], F32R, tag="t")
                                    nc.tensor.transpose(
                                        tps[:], vt[:, s * 128:(s + 1) * 128],
                                        ident_sb[:])
                                    kbg = half * 8 + tt * 4 + s
                                    base = p * VSTRIDE + kbg * 130
                                    nc.vector.tensor_copy(
                                        vnat[:, base:base + 64],
                                        tps[:, 0:64])
                                    nc.vector.tensor_copy(
                                        vnat[:, base + 65:base + 129],
                                        tps[:, 64:128])

        # ---------------- Phase B: attention --------------------------
        with ExitStack() as actx:
            spool = actx.enter_context(
                tc.tile_pool(name="s", bufs=2, space="PSUM"))
            ypool = actx.enter_context(
                tc.tile_pool(name="y", bufs=1, space="PSUM"))
            ppool = actx.enter_context(tc.tile_pool(name="p", bufs=4))
            rpool = actx.enter_context(tc.tile_pool(name="r", bufs=2))
            rbpool = actx.enter_context(
                tc.tile_pool(name="rb", bufs=1, space="PSUM"))
            rbspool = actx.enter_context(tc.tile_pool(name="rbs", bufs=2))
            okpool = actx.enter_context(tc.tile_pool(name="onesk", bufs=1))
            ones_k1 = okpool.tile([1, 128], F32R, tag="ok")
            nc.sync.dma_start(ones_k1[:], ones[0:1, 0:128])

            pending = [None]

            def _normalize(p, qt, y0, y1):
                d0 = rpool.tile([1, 512], F32R, tag="d0")
                d1 = rpool.tile([1, 512], F32R, tag="d1")
                nc.vector.tensor_copy(d0[:], y0[64:65, :])
                nc.vector.tensor_copy(d1[:], y1[64:65, :])
                rb0 = rbpool.tile([64, 512], F32, tag="rb0")
                rb1 = rbpool.tile([64, 512], F32, tag="rb1")
                nc.tensor.matmul(rb0[:], lhsT=ones_k1[:, 0:64],
                                 rhs=d0[:], start=True, stop=True)
                nc.tensor.matmul(rb1[:], lhsT=ones_k1[:, 0:64],
                                 rhs=d1[:], start=True, stop=True)
                rbs = rbspool.tile([128, 512], F32, tag="rbs")
                nc.vector.reciprocal(rbs[0:64, :], rb0[:])
                nc.vector.reciprocal(rbs[64:128, :], rb1[:])
                ycol = p * T + qt * 512
                nc.vector.tensor_mul(ynorm[0:64, ycol:ycol + 512],
                                     y0[0:64, :], rbs[0:64, :])
                nc.vector.tensor_mul(ynorm[64:128, ycol:ycol + 512],
                                     y1[0:64, :], rbs[64:128, :])

            for p in range(PAIRS):
                for qt in range(NQT):
                    nkb = (qt + 1) * 4
                    y0 = ypool.tile([65, 512], F32, tag="y0")
                    y1 = ypool.tile([65, 512], F32, tag="y1")
                    for kb in range(nkb):
                        o = kb - qt * 4
                        scol = max(0, o * 128)
                        width = 512 - scol
                        qcol = p * T + qt * 512 + scol
                        kcol = p * T + kb * 128
                        vbase = p * VSTRIDE + kb * 130
                        # both heads' scores in one 2-bank PSUM tile so a
                        # single ACT instruction exponentiates both
                        s01 = spool.tile([128, 1024], F32, tag="s01")
                        nc.tensor.matmul(
                            s01[:, 0:width],
                            lhsT=(kT[0:64, kcol:kcol + 128]),
                            rhs=(qT[0:64, qcol:qcol + width]),
                            start=True, stop=True)
                        nc.tensor.matmul(
                            s01[:, 512:512 + width],
                            lhsT=(kT[64:128, kcol:kcol + 128]),
                            rhs=(qT[64:128, qcol:qcol + width]),
                            start=True, stop=True)
                        p01 = ppool.tile([128, 1024], F32R, tag="p01")
                        sview = s01[:].rearrange("r (h x) -> r h x",
                                                 h=2)[:, :, 0:width]
                        pview = p01[:].rearrange("r (h x) -> r h x",
                                                 h=2)[:, :, 0:width]
                        nc.scalar.activation(
                            pview, sview, mybir.ActivationFunctionType.Exp)
                        if o >= 0:
                            nc.vector.tensor_mul(p01[:, 0:128],
                                                 p01[:, 0:128], tri_sb[:])
                            nc.vector.tensor_mul(p01[:, 512:640],
                                                 p01[:, 512:640], tri_sb[:])
                        nc.tensor.matmul(
                            y0[:, scol:512],
                            lhsT=(vnat[:, vbase:vbase + 65]),
                            rhs=(p01[:, 0:width]),
                            start=(kb == 0), stop=(kb == nkb - 1))
                        nc.tensor.matmul(
                            y1[:, scol:512],
                            lhsT=(vnat[:, vbase + 65:vbase + 130]),
                            rhs=(p01[:, 512:512 + width]),
                            start=(kb == 0), stop=(kb == nkb - 1))
                        if kb == 1 and pending[0] is not None:
                            pending[0]()
                            pending[0] = None
                    if pending[0] is not None:
                        pending[0]()
                    pending[0] = (lambda p=p, qt=qt, y0=y0, y1=y1:
                                  _normalize(p, qt, y0, y1))

            if pending[0] is not None:
                pending[0]()
                pending[0] = None

        # ---------------- Phase C: o_proj ------------------------------
        with ExitStack() as actx:
            wopool = actx.enter_context(tc.tile_pool(name="wo", bufs=1))
            opspool = actx.enter_context(
                tc.tile_pool(name="ops", bufs=4, space="PSUM"))
            outpool = actx.enter_context(tc.tile_pool(name="osb", bufs=3))
            wo_sb = []
            for p in range(PAIRS):
                w = wopool.tile([128, 1024], F32R, tag=f"wo{p}")
                nc.sync.dma_start(w[:], woT[p * 128:(p + 1) * 128, :])
                wo_sb.append(w)
            for tt in range(T // 128):
                osb = outpool.tile([128, 1024], F32, tag="osb")
                for n in range(2):
                    ps = opspool.tile([128, 512], F32, tag="ops")
                    for p in range(PAIRS):
                        nc.tensor.matmul(
                            ps[:],
                            lhsT=(ynorm[:, p * T + tt * 128:
                                          p * T + tt * 128 + 128]),
                            rhs=(wo_sb[p][:, n * 512:(n + 1) * 512]),
                            start=(p == 0), stop=(p == PAIRS - 1))
                    nc.vector.tensor_copy(osb[:, n * 512:(n + 1) * 512],
                                          ps[:])
                nc.sync.dma_start(out[tt * 128:(tt + 1) * 128, :], osb[:])


def shard_inputs(x, Wq, Wk, Wv, Wo):
    """Returns in_maps for cores 0..7 (core c: batch c//2, group c%2)."""
    x = np.ascontiguousarray(np.asarray(x, np.float32))
    tri = np.triu(np.ones((128, 128), np.float32))  # tri[r,j]=1 iff j>=r
    ident = np.eye(128, dtype=np.float32)
    in_maps = []
    perms = []
    for g in range(GROUPS):
        perm = np.array([(g * HPG + 2 * p + (q >= 64)) * 64 + (q % 64)
                         for p in range(PAIRS) for q in range(128)])
        perms.append(perm)
    w_cache = {}
    for g in range(GROUPS):
        perm = perms[g]
        w_cache[g] = {
            "wqT": np.ascontiguousarray(np.asarray(Wq, np.float32).T[:, perm]),
            "wkT": np.ascontiguousarray(np.asarray(Wk, np.float32).T[:, perm]),
            "wvT": np.ascontiguousarray(np.asarray(Wv, np.float32).T[:, perm]),
            "woT": np.ascontiguousarray(np.asarray(Wo, np.float32).T[perm, :]),
        }
    for c in range(N_CORES):
        b, g = c // 2, c % 2
        in_maps.append({
            "xT": np.ascontiguousarray(x[b].T),
            "tri": tri, "ident": ident,
            "ones": np.ones((128, 128), np.float32), **w_cache[g],
        })
    return in_maps


def kernel(x, Wq, Wk, Wv, Wo):
    nc = build_nc()
    in_maps = shard_inputs(x, Wq, Wk, Wv, Wo)
    res = run_bass_kernel_spmd(nc, in_maps, list(range(N_CORES)))
    out = np.empty((B, T, D), np.float32)
    for b in range(B):
        out[b] = res.results[2 * b]["out"] + res.results[2 * b + 1]["out"]
    return out



# revision 7
# speedup vs baseline: 1.2464x; 1.2464x over previous
"""Causal self-attention (B=4, T=2048, D=1024, H=16) on 8 TRN2 NeuronCores.

Sharding: 2D (batch x head-group). Core c handles batch b = c//2 and head
group g = c%2 (8 heads, processed as 4 pairs).

v2 layout strategy (per core):
  - x is passed pre-transposed from host: xT [D, T].
  - Q/K projections produce qT/kT [128 local dims, T] with head pair 2p/2p+1
    stacked on partitions 0-63 / 64-127; the 1/sqrt(dh) scale is folded into
    the Wq weights on the host.
  - V is projected directly into natural [token, dim] layout (lhsT = x chunk,
    rhs = WvT) -- no PE transposes -- stored bf16 with a ones column per head
    so the PV matmul also accumulates the softmax denominator.
  - Scores are computed transposed: S^T [keys, queries]; causal masking is an
    accumulating PE matmul that adds -38 to masked positions BEFORE exp
    (start/stop accumulation group), so no vector mask multiply is needed.
  - exp() runs without max-subtraction (scores ~N(0,1), fp32 exp safe);
    output P^T is bf16, feeding the PV matmul directly.
  - Softmax denominators: row 64 of the y PSUM tiles -> 1/d via the fast
    custom-DVE reciprocal (reciprocal_approx_fast), broadcast to 64
    partitions with two col-tiled K=1 matmuls, one multiply per head.
  - o_proj consumes ynorm (bf16) as stationary; Wo is bf16; each core emits
    a partial [T, D] product over its 512 local head dims; host sums the two
    partials per batch.

Emission order interleaves projection / attention / o_proj so the Tile
scheduler can fill TensorE gaps during ACT-bound attention stretches and
keep the PE HAM-warm.
"""

import os
import sys

import numpy as np

if not any(os.path.isdir(os.path.join(p, "concourse")) for p in sys.path):
    sys.path.insert(0, "/opt/trn_rl_repo")

import concourse.mybir as mybir
import concourse.tile as tile
from concourse import bacc
from concourse.bass_utils import run_bass_kernel_spmd

B, T, D, H, DH = 4, 2048, 1024, 16, 64
N_CORES = 8
GROUPS = 2          # head groups (tensor-parallel dim)
HPG = H // GROUPS   # heads per group/core
PAIRS = HPG // 2    # head pairs per core
NKB = T // 128      # 128-key blocks per batch
NQT = T // 512      # 512-query tiles per batch
VSTRIDE = NKB * 130 # vnat cols per pair: 16 blocks x [64 dims|1|64 dims|1]

F32 = mybir.dt.float32
F32R = mybir.dt.float32r
BF16 = mybir.dt.bfloat16


def build_nc():
    nc = bacc.Bacc("TRN2", target_bir_lowering=False, debug=False,
                   num_devices=N_CORES)
    xT = nc.dram_tensor("xT", [D, T], F32R, kind="ExternalInput").ap()
    wqT = nc.dram_tensor("wqT", [D, 512], F32R, kind="ExternalInput").ap()
    wkT = nc.dram_tensor("wkT", [D, 512], F32R, kind="ExternalInput").ap()
    wvT = nc.dram_tensor("wvT", [D, 512], F32R, kind="ExternalInput").ap()
    woT = nc.dram_tensor("woT", [512, D], BF16, kind="ExternalInput").ap()
    maskR = nc.dram_tensor("maskR", [128, 256], F32R, kind="ExternalInput").ap()
    identR = nc.dram_tensor("identR", [128, 128], F32R, kind="ExternalInput").ap()
    onesb = nc.dram_tensor("onesb", [128, 128], BF16, kind="ExternalInput").ap()
    sel0 = nc.dram_tensor("sel0", [1, 128], F32R, kind="ExternalInput").ap()
    sel1 = nc.dram_tensor("sel1", [1, 128], F32R, kind="ExternalInput").ap()
    out = nc.dram_tensor("out", [T, D], F32, kind="ExternalOutput").ap()

    with tile.TileContext(nc) as tc:
        _body(tc, out, xT, wqT, wkT, wvT, woT, maskR, identR, onesb, sel0, sel1)
    nc.compile()
    return nc


def _body(tc, out, xT, wqT, wkT, wvT, woT, maskR, identR, onesb, sel0, sel1):
    nc = tc.nc
    from contextlib import ExitStack

    with ExitStack() as ctx:
        persist = ctx.enter_context(tc.tile_pool(name="persist", bufs=1))
        qT = persist.tile([128, PAIRS * T], F32R, tag="qT")
        kT = persist.tile([128, PAIRS * T], F32R, tag="kT")
        vnat = persist.tile([128, PAIRS * VSTRIDE], BF16, tag="vnat")
        ynorm = persist.tile([128, PAIRS * T], BF16, tag="ynorm")

        consts = ctx.enter_context(tc.tile_pool(name="consts", bufs=1))
        mask2_sb = consts.tile([128, 256], F32R, tag="mask")
        nc.sync.dma_start(mask2_sb[:], maskR[:])
        ident_sb = consts.tile([128, 128], F32R, tag="ident")
        nc.sync.dma_start(ident_sb[:], identR[:])
        sel0_sb = consts.tile([1, 128], F32R, tag="sel0")
        nc.sync.dma_start(sel0_sb[:], sel0[:])
        sel1_sb = consts.tile([1, 128], F32R, tag="sel1")
        nc.sync.dma_start(sel1_sb[:], sel1[:])

        # ones columns of vnat (denominator accumulators): col 64 of each
        # 65-wide [dims|1] half-chunk, DMA'd from a DRAM bf16 constant.
        ones_view = vnat[:].rearrange("r (p k m x) -> r (p k m) x",
                                      p=PAIRS, k=NKB, m=2)[:, :, 64:65]
        nc.sync.dma_start(ones_view.squeeze(), onesb[:])

        wpool = ctx.enter_context(tc.tile_pool(name="w", bufs=2))
        wvpool = ctx.enter_context(tc.tile_pool(name="wv", bufs=1))
        wv_sb = wvpool.tile([128, 8 * 512], F32R, tag="wv")
        for c in range(8):
            nc.sync.dma_start(wv_sb[:, c * 512:(c + 1) * 512],
                              wvT[c * 128:(c + 1) * 128, :])
        wopool = ctx.enter_context(tc.tile_pool(name="wo", bufs=1))
        wo_sb = []
        for p in range(PAIRS):
            w = wopool.tile([128, 1024], BF16, tag=f"wo{p}")
            nc.sync.dma_start(w[:], woT[p * 128:(p + 1) * 128, :])
            wo_sb.append(w)

        xpool = ctx.enter_context(tc.tile_pool(name="xt", bufs=1))
        ppool = ctx.enter_context(tc.tile_pool(name="p", bufs=4))
        rpool = ctx.enter_context(tc.tile_pool(name="r", bufs=2))
        opool = ctx.enter_context(tc.tile_pool(name="osb", bufs=3))

        spool = ctx.enter_context(
            tc.tile_pool(name="s", bufs=2, space="PSUM"))
        ypool = ctx.enter_context(
            tc.tile_pool(name="y", bufs=1, space="PSUM"))
        shpool = ctx.enter_context(
            tc.tile_pool(name="sh", bufs=2, space="PSUM"))

        # ---------------- phase emitters -----------------------------
        def proj_half(half):
            x_sb = []
            for c in range(8):
                xt = xpool.tile([128, 1024], F32R, tag=f"x{c}")
                nc.sync.dma_start(
                    xt[:], xT[c * 128:(c + 1) * 128,
                              half * 1024:(half + 1) * 1024])
                x_sb.append(xt)
            for kind, wap, dst in (("q", wqT, qT), ("k", wkT, kT)):
                for p in range(PAIRS):
                    w_sb = wpool.tile([128, 1024], F32R, tag="w")
                    wsrc = wap[:, p * 128:(p + 1) * 128]
                    nc.sync.dma_start(
                        w_sb[:].rearrange("r (c o) -> r c o", o=128),
                        wsrc.rearrange("(c r) o -> r c o", r=128))
                    for tt in range(2):
                        ps = shpool.tile([128, 512], F32, tag="ps")
                        for c in range(8):
                            nc.tensor.matmul(
                                ps[:],
                                lhsT=(w_sb[:, c * 128:(c + 1) * 128]),
                                rhs=(x_sb[c][:, tt * 512:(tt + 1) * 512]),
                                start=(c == 0), stop=(c == 7))
                        col0 = p * T + half * 1024 + tt * 512
                        nc.vector.tensor_copy(dst[:, col0:col0 + 512], ps[:])
            # V in natural [token, dim] layout: 128-token blocks
            for tb in range(8):
                ps = shpool.tile([128, 512], F32, tag="ps")
                for c in range(8):
                    nc.tensor.matmul(
                        ps[:],
                        lhsT=(x_sb[c][:, tb * 128:(tb + 1) * 128]),
                        rhs=(wv_sb[:, c * 512:(c + 1) * 512]),
                        start=(c == 0), stop=(c == 7))
                kb = half * 8 + tb
                src = ps[:].rearrange("r (p m x) -> r p m x", p=PAIRS, m=2)
                dstv = vnat[:].rearrange(
                    "r (p k m x) -> r p k m x",
                    p=PAIRS, k=NKB, m=2)[:, :, kb:kb + 1, :, 0:64]
                nc.vector.tensor_copy(dstv.squeeze(2), src)

        pending = [None]

        def _normalize(p, qt, y0, y1):
            den0 = rpool.tile([1, 512], F32R, tag="den0")
            den1 = rpool.tile([1, 512], F32R, tag="den1")
            nc.vector.tensor_copy(den0[:], y0[64:65, :])
            nc.vector.tensor_copy(den1[:], y1[64:65, :])
            # broadcast raw denominators to 64 partitions each (col-tiled
            # K=1 matmuls), then one fast-approx reciprocal on the full
            # [128, 512] tile (DVE cost is free-dim-bound, so this is as
            # cheap as a [1, 512] reciprocal)
            rbs = shpool.tile([128, 512], F32, tag="ps")
            nc.tensor.matmul(rbs[:], lhsT=sel0_sb[:],
                             rhs=den0[:], start=True, stop=False)
            nc.tensor.matmul(rbs[:], lhsT=sel1_sb[:],
                             rhs=den1[:], start=False, stop=True)
            rcp = rpool.tile([128, 512], F32, tag="rcp")
            nc.vector.reciprocal_approx_fast(out=rcp[:], in_=rbs[:])
            ycol = p * T + qt * 512
            nc.vector.tensor_mul(ynorm[0:64, ycol:ycol + 512],
                                 y0[0:64, :], rcp[0:64, :])
            nc.vector.tensor_mul(ynorm[64:128, ycol:ycol + 512],
                                 y1[0:64, :], rcp[64:128, :])

        def attn_qt(qt):
            nkb = (qt + 1) * 4
            for p in range(PAIRS):
                y0 = ypool.tile([65, 512], F32, tag="y0")
                y1 = ypool.tile([65, 512], F32, tag="y1")
                for kb in range(nkb):
                    o = kb - qt * 4
                    scol = max(0, o * 128)
                    width = 512 - scol
                    qcol = p * T + qt * 512 + scol
                    kcol = p * T + kb * 128
                    vbase = p * VSTRIDE + kb * 130
                    # both heads' scores in one 2-bank PSUM tile so a
                    # single ACT instruction exponentiates both
                    s01 = spool.tile([128, 1024], F32, tag="s01")
                    nc.tensor.matmul(
                        s01[:, 0:width],
                        lhsT=(kT[0:64, kcol:kcol + 128]),
                        rhs=(qT[0:64, qcol:qcol + width]),
                        start=True, stop=(o < 0))
                    nc.tensor.matmul(
                        s01[:, 512:512 + width],
                        lhsT=(kT[64:128, kcol:kcol + 128]),
                        rhs=(qT[64:128, qcol:qcol + width]),
                        start=True, stop=(o < 0))
                    if o >= 0:
                        # causal mask: accumulate -38 into masked positions
                        # of the diagonal 128-col chunk (both heads in one
                        # N=256 matmul: fp32r needs N>=256 for full rate)
                        mview = s01[:].rearrange("r (h x) -> r h x",
                                                 h=2)[:, :, 0:128]
                        nc.tensor.matmul(
                            mview, lhsT=ident_sb[:],
                            rhs=mask2_sb[:].rearrange("r (h x) -> r h x",
                                                      h=2),
                            start=False, stop=True)
                    p01 = ppool.tile([128, 1024], BF16, tag="p01")
                    sview = s01[:].rearrange("r (h x) -> r h x",
                                             h=2)[:, :, 0:width]
                    pview = p01[:].rearrange("r (h x) -> r h x",
                                             h=2)[:, :, 0:width]
                    nc.scalar.activation(
                        pview, sview, mybir.ActivationFunctionType.Exp)
                    nc.tensor.matmul(
                        y0[:, scol:512],
                        lhsT=(vnat[:, vbase:vbase + 65]),
                        rhs=(p01[:, 0:width]),
                        start=(kb == 0), stop=(kb == nkb - 1))
                    nc.tensor.matmul(
                        y1[:, scol:512],
                        lhsT=(vnat[:, vbase + 65:vbase + 130]),
                        rhs=(p01[:, 512:512 + width]),
                        start=(kb == 0), stop=(kb == nkb - 1))
                    if kb == 1 and pending[0] is not None:
                        pending[0]()
                        pending[0] = None
                if pending[0] is not None:
                    pending[0]()
                pending[0] = (lambda p=p, qt=qt, y0=y0, y1=y1:
                              _normalize(p, qt, y0, y1))

        def oproj_qt(qt):
            # flush the last pair's normalize so ynorm for qt is complete
            if pending[0] is not None:
                pending[0]()
                pending[0] = None
            for tt in range(qt * 4, qt * 4 + 4):
                osb = opool.tile([128, 1024], F32, tag="osb")
                for n in range(2):
                    ps = shpool.tile([128, 512], F32, tag="ps")
                    for p in range(PAIRS):
                        nc.tensor.matmul(
                            ps[:],
                            lhsT=(ynorm[:, p * T + tt * 128:
                                          p * T + tt * 128 + 128]),
                            rhs=(wo_sb[p][:, n * 512:(n + 1) * 512]),
                            start=(p == 0), stop=(p == PAIRS - 1))
                    nc.vector.tensor_copy(osb[:, n * 512:(n + 1) * 512],
                                          ps[:])
                nc.sync.dma_start(out[tt * 128:(tt + 1) * 128, :], osb[:])

        # ---------------- emission order -----------------------------
        proj_half(0)
        attn_qt(0)
        oproj_qt(0)
        proj_half(1)
        attn_qt(1)
        oproj_qt(1)
        attn_qt(2)
        oproj_qt(2)
        attn_qt(3)
        oproj_qt(3)


def shard_inputs(x, Wq, Wk, Wv, Wo):
    """Returns in_maps for cores 0..7 (core c: batch c//2, group c%2)."""
    import ml_dtypes
    x = np.ascontiguousarray(np.asarray(x, np.float32))
    mask1 = np.zeros((128, 128), np.float32)
    for r in range(128):
        mask1[r, :r] = -38.0  # S^T[key r, query j]: masked iff j < r
    mask = np.ascontiguousarray(np.concatenate([mask1, mask1], axis=1))
    ident = np.eye(128, dtype=np.float32)
    sel0 = np.zeros((1, 128), np.float32); sel0[0, :64] = 1.0
    sel1 = np.zeros((1, 128), np.float32); sel1[0, 64:] = 1.0
    in_maps = []
    perms = []
    for g in range(GROUPS):
        perm = np.array([(g * HPG + 2 * p + (q >= 64)) * 64 + (q % 64)
                         for p in range(PAIRS) for q in range(128)])
        perms.append(perm)
    w_cache = {}
    qscale = 1.0 / np.sqrt(DH)
    for g in range(GROUPS):
        perm = perms[g]
        w_cache[g] = {
            "wqT": np.ascontiguousarray(
                (np.asarray(Wq, np.float32).T * qscale)[:, perm]),
            "wkT": np.ascontiguousarray(np.asarray(Wk, np.float32).T[:, perm]),
            "wvT": np.ascontiguousarray(np.asarray(Wv, np.float32).T[:, perm]),
            "woT": np.ascontiguousarray(
                np.asarray(Wo, np.float32).T[perm, :]).astype(
                    ml_dtypes.bfloat16),
        }
    for c in range(N_CORES):
        b, g = c // 2, c % 2
        in_maps.append({
            "xT": np.ascontiguousarray(x[b].T),
            "maskR": mask, "identR": ident,
            "onesb": np.ones((128, 128), ml_dtypes.bfloat16),
            "sel0": sel0, "sel1": sel1,
            **w_cache[g],
        })
    return in_maps


def kernel(x, Wq, Wk, Wv, Wo):
    nc = build_nc()
    in_maps = shard_inputs(x, Wq, Wk, Wv, Wo)
    res = run_bass_kernel_spmd(nc, in_maps, list(range(N_CORES)))
    out = np.empty((B, T, D), np.float32)
    for b in range(B):
        out[b] = res.results[2 * b]["out"] + res.results[2 * b + 1]["out"]
    return out


# revision 10
# speedup vs baseline: 1.2638x; 1.0139x over previous
"""Causal self-attention (B=4, T=2048, D=1024, H=16) on 8 TRN2 NeuronCores.

Sharding: 2D (batch x head-group). Core c handles batch b = c//2 and head
group g = c%2 (8 heads, processed as 4 pairs).

v2 layout strategy (per core):
  - x is passed pre-transposed from host: xT [D, T].
  - Q/K projections produce qT/kT [128 local dims, T] with head pair 2p/2p+1
    stacked on partitions 0-63 / 64-127; the 1/sqrt(dh) scale is folded into
    the Wq weights on the host.
  - V is projected directly into natural [token, dim] layout (lhsT = x chunk,
    rhs = WvT) -- no PE transposes -- stored bf16 with a ones column per head
    so the PV matmul also accumulates the softmax denominator.
  - Scores are computed transposed: S^T [keys, queries]; causal masking is an
    accumulating PE matmul that adds -38 to masked positions BEFORE exp
    (start/stop accumulation group), so no vector mask multiply is needed.
  - exp() runs without max-subtraction (scores ~N(0,1), fp32 exp safe);
    output P^T is bf16, feeding the PV matmul directly.
  - Softmax denominators: row 64 of the y PSUM tiles -> 1/d via the fast
    custom-DVE reciprocal (reciprocal_approx_fast), broadcast to 64
    partitions with two col-tiled K=1 matmuls, one multiply per head.
  - o_proj consumes ynorm (bf16) as stationary; Wo is bf16; each core emits
    a partial [T, D] product over its 512 local head dims; host sums the two
    partials per batch.

Emission order interleaves projection / attention / o_proj so the Tile
scheduler can fill TensorE gaps during ACT-bound attention stretches and
keep the PE HAM-warm.
"""

import os
import sys

import numpy as np

if not any(os.path.isdir(os.path.join(p, "concourse")) for p in sys.path):
    sys.path.insert(0, "/opt/trn_rl_repo")

import concourse.mybir as mybir
import concourse.tile as tile
from concourse import bacc
from concourse.bass_utils import run_bass_kernel_spmd

B, T, D, H, DH = 4, 2048, 1024, 16, 64
N_CORES = 8
GROUPS = 2          # head groups (tensor-parallel dim)
HPG = H // GROUPS   # heads per group/core
PAIRS = HPG // 2    # head pairs per core
NKB = T // 128      # 128-key blocks per batch
NQT = T // 512      # 512-query tiles per batch
VSTRIDE = NKB * 130 # vnat cols per pair: 16 blocks x [64 dims|1|64 dims|1]

F32 = mybir.dt.float32
F32R = mybir.dt.float32r
BF16 = mybir.dt.bfloat16


def build_nc():
    nc = bacc.Bacc("TRN2", target_bir_lowering=False, debug=False,
                   num_devices=N_CORES)
    xT = nc.dram_tensor("xT", [D, T], F32R, kind="ExternalInput").ap()
    wqR = nc.dram_tensor("wqR", [128, 4096], F32R, kind="ExternalInput").ap()
    wkR = nc.dram_tensor("wkR", [128, 4096], F32R, kind="ExternalInput").ap()
    wvT = nc.dram_tensor("wvT", [D, 512], F32R, kind="ExternalInput").ap()
    woT = nc.dram_tensor("woT", [512, D], BF16, kind="ExternalInput").ap()
    maskR = nc.dram_tensor("maskR", [128, 256], F32R, kind="ExternalInput").ap()
    identR = nc.dram_tensor("identR", [128, 128], F32R, kind="ExternalInput").ap()
    onesb = nc.dram_tensor("onesb", [128, 128], BF16, kind="ExternalInput").ap()
    sel0 = nc.dram_tensor("sel0", [1, 128], F32R, kind="ExternalInput").ap()
    sel1 = nc.dram_tensor("sel1", [1, 128], F32R, kind="ExternalInput").ap()
    out = nc.dram_tensor("out", [T, D], F32, kind="ExternalOutput").ap()

    with tile.TileContext(nc) as tc:
        _body(tc, out, xT, wqR, wkR, wvT, woT, maskR, identR, onesb, sel0, sel1)
    nc.compile()
    return nc


def _body(tc, out, xT, wqR, wkR, wvT, woT, maskR, identR, onesb, sel0, sel1):
    nc = tc.nc
    from contextlib import ExitStack

    with ExitStack() as ctx:
        persist = ctx.enter_context(tc.tile_pool(name="persist", bufs=1))
        qT = persist.tile([128, PAIRS * T], F32R, tag="qT")
        kT = persist.tile([128, PAIRS * T], F32R, tag="kT")
        vnat = persist.tile([128, PAIRS * VSTRIDE], BF16, tag="vnat")
        ynorm = persist.tile([128, PAIRS * T], BF16, tag="ynorm")

        consts = ctx.enter_context(tc.tile_pool(name="consts", bufs=1))
        mask2_sb = consts.tile([128, 256], F32R, tag="mask")
        nc.sync.dma_start(mask2_sb[:], maskR[:])
        ident_sb = consts.tile([128, 128], F32R, tag="ident")
        nc.sync.dma_start(ident_sb[:], identR[:])
        sel0_sb = consts.tile([1, 128], F32R, tag="sel0")
        nc.sync.dma_start(sel0_sb[:], sel0[:])
        sel1_sb = consts.tile([1, 128], F32R, tag="sel1")
        nc.sync.dma_start(sel1_sb[:], sel1[:])

        # ones columns of vnat (denominator accumulators): col 64 of each
        # 65-wide [dims|1] half-chunk, DMA'd from a DRAM bf16 constant.
        ones_view = vnat[:].rearrange("r (p k m x) -> r (p k m) x",
                                      p=PAIRS, k=NKB, m=2)[:, :, 64:65]
        nc.sync.dma_start(ones_view.squeeze(), onesb[:])

        wqkpool = ctx.enter_context(tc.tile_pool(name="wqk", bufs=1))
        wq_sb = wqkpool.tile([128, 4096], F32R, tag="wq")
        nc.sync.dma_start(wq_sb[:], wqR[:])
        wk_sb = wqkpool.tile([128, 4096], F32R, tag="wk")
        nc.sync.dma_start(wk_sb[:], wkR[:])
        wvpool = ctx.enter_context(tc.tile_pool(name="wv", bufs=1))
        wv_sb = wvpool.tile([128, 8 * 512], F32R, tag="wv")
        for c in range(8):
            nc.sync.dma_start(wv_sb[:, c * 512:(c + 1) * 512],
                              wvT[c * 128:(c + 1) * 128, :])
        wopool = ctx.enter_context(tc.tile_pool(name="wo", bufs=1))
        wo_sb = []
        for p in range(PAIRS):
            w = wopool.tile([128, 1024], BF16, tag=f"wo{p}")
            nc.sync.dma_start(w[:], woT[p * 128:(p + 1) * 128, :])
            wo_sb.append(w)

        xpool = ctx.enter_context(tc.tile_pool(name="xt", bufs=1))
        ppool = ctx.enter_context(tc.tile_pool(name="p", bufs=3))
        rpool = ctx.enter_context(tc.tile_pool(name="r", bufs=1))
        opool = ctx.enter_context(tc.tile_pool(name="osb", bufs=2))

        spool = ctx.enter_context(
            tc.tile_pool(name="s", bufs=2, space="PSUM"))
        ypool = ctx.enter_context(
            tc.tile_pool(name="y", bufs=1, space="PSUM"))
        shpool = ctx.enter_context(
            tc.tile_pool(name="sh", bufs=2, space="PSUM"))

        # ---------------- phase emitters -----------------------------
        x_sb = [None] * 8

        def load_x(half):
            for c in range(8):
                xt = xpool.tile([128, 1024], F32R, tag=f"x{c}")
                nc.sync.dma_start(
                    xt[:], xT[c * 128:(c + 1) * 128,
                              half * 1024:(half + 1) * 1024])
                x_sb[c] = xt

        def proj_chunk(half, sub):
            # q/k for 512 tokens
            for w_sb, dst in ((wq_sb, qT), (wk_sb, kT)):
                for p in range(PAIRS):
                    ps = shpool.tile([128, 512], F32, tag="ps")
                    for c in range(8):
                        nc.tensor.matmul(
                            ps[:],
                            lhsT=(w_sb[:, p * 1024 + c * 128:
                                         p * 1024 + (c + 1) * 128]),
                            rhs=(x_sb[c][:, sub * 512:(sub + 1) * 512]),
                            start=(c == 0), stop=(c == 7))
                    col0 = p * T + half * 1024 + sub * 512
                    nc.vector.tensor_copy(dst[:, col0:col0 + 512], ps[:])
            # V in natural [token, dim] layout: 128-token blocks
            for tb in range(4):
                ps = shpool.tile([128, 512], F32, tag="ps")
                tok0 = sub * 512 + tb * 128
                for c in range(8):
                    nc.tensor.matmul(
                        ps[:],
                        lhsT=(x_sb[c][:, tok0:tok0 + 128]),
                        rhs=(wv_sb[:, c * 512:(c + 1) * 512]),
                        start=(c == 0), stop=(c == 7))
                kb = half * 8 + sub * 4 + tb
                srcv = ps[:].rearrange("r (p m x) -> r p m x", p=PAIRS, m=2)
                dstv = vnat[:].rearrange(
                    "r (p k m x) -> r p k m x",
                    p=PAIRS, k=NKB, m=2)[:, :, kb:kb + 1, :, 0:64]
                nc.vector.tensor_copy(dstv.squeeze(2), srcv)

        pending = [None]

        def _normalize(p, qt, y0, y1):
            den0 = rpool.tile([1, 512], F32R, tag="den0")
            den1 = rpool.tile([1, 512], F32R, tag="den1")
            nc.vector.tensor_copy(den0[:], y0[64:65, :])
            nc.vector.tensor_copy(den1[:], y1[64:65, :])
            # broadcast raw denominators to 64 partitions each (col-tiled
            # K=1 matmuls), then one fast-approx reciprocal on the full
            # [128, 512] tile (DVE cost is free-dim-bound, so this is as
            # cheap as a [1, 512] reciprocal)
            rbs = shpool.tile([128, 512], F32, tag="ps")
            nc.tensor.matmul(rbs[:], lhsT=sel0_sb[:],
                             rhs=den0[:], start=True, stop=False)
            nc.tensor.matmul(rbs[:], lhsT=sel1_sb[:],
                             rhs=den1[:], start=False, stop=True)
            rcp = rpool.tile([128, 512], F32, tag="rcp")
            nc.vector.reciprocal_approx_fast(out=rcp[:], in_=rbs[:])
            ycol = p * T + qt * 512
            nc.vector.tensor_mul(ynorm[0:64, ycol:ycol + 512],
                                 y0[0:64, :], rcp[0:64, :])
            nc.vector.tensor_mul(ynorm[64:128, ycol:ycol + 512],
                                 y1[0:64, :], rcp[64:128, :])

        def attn_qt(qt):
            nkb = (qt + 1) * 4
            for p in range(PAIRS):
                y0 = ypool.tile([65, 512], F32, tag="y0")
                y1 = ypool.tile([65, 512], F32, tag="y1")
                for kb in range(nkb):
                    o = kb - qt * 4
                    scol = max(0, o * 128)
                    width = 512 - scol
                    qcol = p * T + qt * 512 + scol
                    kcol = p * T + kb * 128
                    vbase = p * VSTRIDE + kb * 130
                    # both heads' scores in one 2-bank PSUM tile so a
                    # single ACT instruction exponentiates both
                    s01 = spool.tile([128, 1024], F32, tag="s01")
                    nc.tensor.matmul(
                        s01[:, 0:width],
                        lhsT=(kT[0:64, kcol:kcol + 128]),
                        rhs=(qT[0:64, qcol:qcol + width]),
                        start=True, stop=(o < 0))
                    nc.tensor.matmul(
                        s01[:, 512:512 + width],
                        lhsT=(kT[64:128, kcol:kcol + 128]),
                        rhs=(qT[64:128, qcol:qcol + width]),
                        start=True, stop=(o < 0))
                    if o >= 0:
                        # causal mask: accumulate -38 into masked positions
                        # of the diagonal 128-col chunk (both heads in one
                        # N=256 matmul: fp32r needs N>=256 for full rate)
                        mview = s01[:].rearrange("r (h x) -> r h x",
                                                 h=2)[:, :, 0:128]
                        nc.tensor.matmul(
                            mview, lhsT=ident_sb[:],
                            rhs=mask2_sb[:].rearrange("r (h x) -> r h x",
                                                      h=2),
                            start=False, stop=True)
                    p01 = ppool.tile([128, 1024], BF16, tag="p01")
                    sview = s01[:].rearrange("r (h x) -> r h x",
                                             h=2)[:, :, 0:width]
                    pview = p01[:].rearrange("r (h x) -> r h x",
                                             h=2)[:, :, 0:width]
                    nc.scalar.activation(
                        pview, sview, mybir.ActivationFunctionType.Exp)
                    nc.tensor.matmul(
                        y0[:, scol:512],
                        lhsT=(vnat[:, vbase:vbase + 65]),
                        rhs=(p01[:, 0:width]),
                        start=(kb == 0), stop=(kb == nkb - 1))
                    nc.tensor.matmul(
                        y1[:, scol:512],
                        lhsT=(vnat[:, vbase + 65:vbase + 130]),
                        rhs=(p01[:, 512:512 + width]),
                        start=(kb == 0), stop=(kb == nkb - 1))
                    if kb == 1 and pending[0] is not None:
                        pending[0]()
                        pending[0] = None
                if pending[0] is not None:
                    pending[0]()
                pending[0] = (lambda p=p, qt=qt, y0=y0, y1=y1:
                              _normalize(p, qt, y0, y1))

        def oproj_qt(qt):
            # flush the last pair's normalize so ynorm for qt is complete
            if pending[0] is not None:
                pending[0]()
                pending[0] = None
            for tt in range(qt * 4, qt * 4 + 4):
                osb = opool.tile([128, 1024], F32, tag="osb")
                for n in range(2):
                    ps = shpool.tile([128, 512], F32, tag="ps")
                    for p in range(PAIRS):
                        nc.tensor.matmul(
                            ps[:],
                            lhsT=(ynorm[:, p * T + tt * 128:
                                          p * T + tt * 128 + 128]),
                            rhs=(wo_sb[p][:, n * 512:(n + 1) * 512]),
                            start=(p == 0), stop=(p == PAIRS - 1))
                    nc.vector.tensor_copy(osb[:, n * 512:(n + 1) * 512],
                                          ps[:])
                nc.sync.dma_start(out[tt * 128:(tt + 1) * 128, :], osb[:])

        # ---------------- emission order -----------------------------
        load_x(0)
        proj_chunk(0, 0)
        attn_qt(0)
        proj_chunk(0, 1)
        oproj_qt(0)
        attn_qt(1)
        load_x(1)
        proj_chunk(1, 0)
        oproj_qt(1)
        attn_qt(2)
        proj_chunk(1, 1)
        oproj_qt(2)
        attn_qt(3)
        oproj_qt(3)


def shard_inputs(x, Wq, Wk, Wv, Wo):
    """Returns in_maps for cores 0..7 (core c: batch c//2, group c%2)."""
    import ml_dtypes
    x = np.ascontiguousarray(np.asarray(x, np.float32))
    mask1 = np.zeros((128, 128), np.float32)
    for r in range(128):
        mask1[r, :r] = -38.0  # S^T[key r, query j]: masked iff j < r
    mask = np.ascontiguousarray(np.concatenate([mask1, mask1], axis=1))
    ident = np.eye(128, dtype=np.float32)
    sel0 = np.zeros((1, 128), np.float32); sel0[0, :64] = 1.0
    sel1 = np.zeros((1, 128), np.float32); sel1[0, 64:] = 1.0
    in_maps = []
    perms = []
    for g in range(GROUPS):
        perm = np.array([(g * HPG + 2 * p + (q >= 64)) * 64 + (q % 64)
                         for p in range(PAIRS) for q in range(128)])
        perms.append(perm)
    w_cache = {}
    qscale = 1.0 / np.sqrt(DH)
    for g in range(GROUPS):
        perm = perms[g]
        wqT = (np.asarray(Wq, np.float32).T * qscale)[:, perm]
        wkT = np.asarray(Wk, np.float32).T[:, perm]
        # [r, p*1024 + c*128 + o] = wT[c*128 + r, p*128 + o]
        def _re(wT):
            w4 = wT.reshape(8, 128, 4, 128)        # [c, r, p, o]
            return np.ascontiguousarray(
                w4.transpose(1, 2, 0, 3).reshape(128, 4096))
        w_cache[g] = {
            "wqR": _re(wqT),
            "wkR": _re(wkT),
            "wvT": np.ascontiguousarray(np.asarray(Wv, np.float32).T[:, perm]),
            "woT": np.ascontiguousarray(
                np.asarray(Wo, np.float32).T[perm, :]).astype(
                    ml_dtypes.bfloat16),
        }
    for c in range(N_CORES):
        b, g = c // 2, c % 2
        in_maps.append({
            "xT": np.ascontiguousarray(x[b].T),
            "maskR": mask, "identR": ident,
            "onesb": np.ones((128, 128), ml_dtypes.bfloat16),
            "sel0": sel0, "sel1": sel1,
            **w_cache[g],
        })
    return in_maps


def kernel(x, Wq, Wk, Wv, Wo):
    nc = build_nc()
    in_maps = shard_inputs(x, Wq, Wk, Wv, Wo)
    res = run_bass_kernel_spmd(nc, in_maps, list(range(N_CORES)))
    out = np.empty((B, T, D), np.float32)
    for b in range(B):
        out[b] = res.results[2 * b]["out"] + res.results[2 * b + 1]["out"]
    return out


# revision 12
# speedup vs baseline: 1.2770x; 1.0104x over previous
"""Causal self-attention (B=4, T=2048, D=1024, H=16) on 8 TRN2 NeuronCores.

Sharding: 2D (batch x head-group). Core c handles batch b = c//2 and head
group g = c%2 (8 heads, processed as 4 pairs).

v2 layout strategy (per core):
  - x is passed pre-transposed from host: xT [D, T].
  - Q/K projections produce qT/kT [128 local dims, T] with head pair 2p/2p+1
    stacked on partitions 0-63 / 64-127; the 1/sqrt(dh) scale is folded into
    the Wq weights on the host.
  - V is projected directly into natural [token, dim] layout (lhsT = x chunk,
    rhs = WvT) -- no PE transposes -- stored bf16 with a ones column per head
    so the PV matmul also accumulates the softmax denominator.
  - Scores are computed transposed: S^T [keys, queries]; causal masking is an
    accumulating PE matmul that adds -38 to masked positions BEFORE exp
    (start/stop accumulation group), so no vector mask multiply is needed.
  - exp() runs without max-subtraction (scores ~N(0,1), fp32 exp safe);
    output P^T is bf16, feeding the PV matmul directly.
  - Softmax denominators: row 64 of the y PSUM tiles -> 1/d via the fast
    custom-DVE reciprocal (reciprocal_approx_fast), broadcast to 64
    partitions with two col-tiled K=1 matmuls, one multiply per head.
  - o_proj consumes ynorm (bf16) as stationary; Wo is bf16; each core emits
    a partial [T, D] product over its 512 local head dims; host sums the two
    partials per batch.

Emission order interleaves projection / attention / o_proj so the Tile
scheduler can fill TensorE gaps during ACT-bound attention stretches and
keep the PE HAM-warm.
"""

import os
import sys

import numpy as np

if not any(os.path.isdir(os.path.join(p, "concourse")) for p in sys.path):
    sys.path.insert(0, "/opt/trn_rl_repo")

import concourse.mybir as mybir
import concourse.tile as tile
from concourse import bacc
from concourse.bass_utils import run_bass_kernel_spmd

B, T, D, H, DH = 4, 2048, 1024, 16, 64
N_CORES = 8
GROUPS = 2          # head groups (tensor-parallel dim)
HPG = H // GROUPS   # heads per group/core
PAIRS = HPG // 2    # head pairs per core
NKB = T // 128      # 128-key blocks per batch
NQT = T // 512      # 512-query tiles per batch
VSTRIDE = NKB * 130 # vnat cols per pair: 16 blocks x [64 dims|1|64 dims|1]

F32 = mybir.dt.float32
F32R = mybir.dt.float32r
BF16 = mybir.dt.bfloat16


def build_nc():
    nc = bacc.Bacc("TRN2", target_bir_lowering=False, debug=False,
                   num_devices=N_CORES)
    xT = nc.dram_tensor("xT", [D, T], F32R, kind="ExternalInput").ap()
    wqR = nc.dram_tensor("wqR", [128, 4096], F32R, kind="ExternalInput").ap()
    wkR = nc.dram_tensor("wkR", [128, 4096], F32R, kind="ExternalInput").ap()
    wvT = nc.dram_tensor("wvT", [D, 512], F32R, kind="ExternalInput").ap()
    woT = nc.dram_tensor("woT", [512, D], BF16, kind="ExternalInput").ap()
    maskR = nc.dram_tensor("maskR", [128, 256], F32R, kind="ExternalInput").ap()
    identR = nc.dram_tensor("identR", [128, 128], F32R, kind="ExternalInput").ap()
    onesb = nc.dram_tensor("onesb", [128, 128], BF16, kind="ExternalInput").ap()
    sel0 = nc.dram_tensor("sel0", [1, 128], F32R, kind="ExternalInput").ap()
    sel1 = nc.dram_tensor("sel1", [1, 128], F32R, kind="ExternalInput").ap()
    out = nc.dram_tensor("out", [T, D], F32, kind="ExternalOutput").ap()

    with tile.TileContext(nc) as tc:
        _body(tc, out, xT, wqR, wkR, wvT, woT, maskR, identR, onesb, sel0, sel1)
    nc.compile()
    return nc


def _body(tc, out, xT, wqR, wkR, wvT, woT, maskR, identR, onesb, sel0, sel1):
    nc = tc.nc
    from contextlib import ExitStack

    with ExitStack() as ctx:
        persist = ctx.enter_context(tc.tile_pool(name="persist", bufs=1))
        qT = persist.tile([128, PAIRS * T], F32R, tag="qT")
        kT = persist.tile([128, PAIRS * T], F32R, tag="kT")
        vnat = persist.tile([128, PAIRS * VSTRIDE], BF16, tag="vnat")
        ynorm = persist.tile([128, PAIRS * T], BF16, tag="ynorm")

        consts = ctx.enter_context(tc.tile_pool(name="consts", bufs=1))
        mask2_sb = consts.tile([128, 256], F32R, tag="mask")
        nc.sync.dma_start(mask2_sb[:], maskR[:])
        ident_sb = consts.tile([128, 128], F32R, tag="ident")
        nc.sync.dma_start(ident_sb[:], identR[:])
        sel0_sb = consts.tile([1, 128], F32R, tag="sel0")
        nc.sync.dma_start(sel0_sb[:], sel0[:])
        sel1_sb = consts.tile([1, 128], F32R, tag="sel1")
        nc.sync.dma_start(sel1_sb[:], sel1[:])

        # ones columns of vnat (denominator accumulators): col 64 of each
        # 65-wide [dims|1] half-chunk, DMA'd from a DRAM bf16 constant.
        ones_view = vnat[:].rearrange("r (p k m x) -> r (p k m) x",
                                      p=PAIRS, k=NKB, m=2)[:, :, 64:65]
        nc.sync.dma_start(ones_view.squeeze(), onesb[:])

        wqkpool = ctx.enter_context(tc.tile_pool(name="wqk", bufs=1))
        wq_sb = wqkpool.tile([128, 4096], F32R, tag="wq")
        wk_sb = wqkpool.tile([128, 4096], F32R, tag="wk")
        wvpool = ctx.enter_context(tc.tile_pool(name="wv", bufs=1))
        wv_sb = wvpool.tile([128, 8 * 512], F32R, tag="wv")
        wopool = ctx.enter_context(tc.tile_pool(name="wo", bufs=1))
        wo_sb = []
        for p in range(PAIRS):
            wot = wopool.tile([128, 1024], BF16, tag=f"wo{p}")
            wo_sb.append(wot)

        xpool = ctx.enter_context(tc.tile_pool(name="xt", bufs=1))
        ppool = ctx.enter_context(tc.tile_pool(name="p", bufs=3))
        rpool = ctx.enter_context(tc.tile_pool(name="r", bufs=1))
        opool = ctx.enter_context(tc.tile_pool(name="osb", bufs=2))

        spool = ctx.enter_context(
            tc.tile_pool(name="s", bufs=2, space="PSUM"))
        ypool = ctx.enter_context(
            tc.tile_pool(name="y", bufs=1, space="PSUM"))
        shpool = ctx.enter_context(
            tc.tile_pool(name="sh", bufs=2, space="PSUM"))

        # ---------------- phase emitters -----------------------------
        x_sb = [None] * 8

        def load_x(half):
            for c in range(8):
                xt = xpool.tile([128, 1024], F32R, tag=f"x{c}")
                nc.sync.dma_start(
                    xt[:], xT[c * 128:(c + 1) * 128,
                              half * 1024:(half + 1) * 1024])
                x_sb[c] = xt

        def prologue_dmas():
            # ordered so the first q-proj matmul can start after ~1MB:
            # wq strip c + x strip c interleaved, then wk, wv, wo
            for c in range(8):
                nc.sync.dma_start(wq_sb[:, c * 512:(c + 1) * 512],
                                  wqR[:, c * 512:(c + 1) * 512])
                xt = xpool.tile([128, 1024], F32R, tag=f"x{c}")
                nc.sync.dma_start(xt[:], xT[c * 128:(c + 1) * 128, 0:1024])
                x_sb[c] = xt
            for c in range(8):
                nc.sync.dma_start(wk_sb[:, c * 512:(c + 1) * 512],
                                  wkR[:, c * 512:(c + 1) * 512])
            for c in range(8):
                nc.sync.dma_start(wv_sb[:, c * 512:(c + 1) * 512],
                                  wvT[c * 128:(c + 1) * 128, :])
            for p in range(PAIRS):
                nc.sync.dma_start(wo_sb[p][:], woT[p * 128:(p + 1) * 128, :])

        def proj_chunk(half, sub):
            # q/k for 512 tokens
            for w_sb, dst in ((wq_sb, qT), (wk_sb, kT)):
                for p in range(PAIRS):
                    ps = shpool.tile([128, 512], F32, tag="ps")
                    for c in range(8):
                        nc.tensor.matmul(
                            ps[:],
                            lhsT=(w_sb[:, c * 512 + p * 128:
                                         c * 512 + (p + 1) * 128]),
                            rhs=(x_sb[c][:, sub * 512:(sub + 1) * 512]),
                            start=(c == 0), stop=(c == 7))
                    col0 = p * T + half * 1024 + sub * 512
                    nc.vector.tensor_copy(dst[:, col0:col0 + 512], ps[:])
            # V in natural [token, dim] layout: 128-token blocks
            for tb in range(4):
                ps = shpool.tile([128, 512], F32, tag="ps")
                tok0 = sub * 512 + tb * 128
                for c in range(8):
                    nc.tensor.matmul(
                        ps[:],
                        lhsT=(x_sb[c][:, tok0:tok0 + 128]),
                        rhs=(wv_sb[:, c * 512:(c + 1) * 512]),
                        start=(c == 0), stop=(c == 7))
                kb = half * 8 + sub * 4 + tb
                srcv = ps[:].rearrange("r (p m x) -> r p m x", p=PAIRS, m=2)
                dstv = vnat[:].rearrange(
                    "r (p k m x) -> r p k m x",
                    p=PAIRS, k=NKB, m=2)[:, :, kb:kb + 1, :, 0:64]
                nc.vector.tensor_copy(dstv.squeeze(2), srcv)

        pending = [None]

        def _normalize(p, qt, y0, y1):
            den0 = rpool.tile([1, 512], F32R, tag="den0")
            den1 = rpool.tile([1, 512], F32R, tag="den1")
            nc.vector.tensor_copy(den0[:], y0[64:65, :])
            nc.vector.tensor_copy(den1[:], y1[64:65, :])
            # broadcast raw denominators to 64 partitions each (col-tiled
            # K=1 matmuls), then one fast-approx reciprocal on the full
            # [128, 512] tile (DVE cost is free-dim-bound, so this is as
            # cheap as a [1, 512] reciprocal)
            rbs = shpool.tile([128, 512], F32, tag="ps")
            nc.tensor.matmul(rbs[:], lhsT=sel0_sb[:],
                             rhs=den0[:], start=True, stop=False)
            nc.tensor.matmul(rbs[:], lhsT=sel1_sb[:],
                             rhs=den1[:], start=False, stop=True)
            rcp = rpool.tile([128, 512], F32, tag="rcp")
            nc.vector.reciprocal_approx_fast(out=rcp[:], in_=rbs[:])
            ycol = p * T + qt * 512
            nc.vector.tensor_mul(ynorm[0:64, ycol:ycol + 512],
                                 y0[0:64, :], rcp[0:64, :])
            nc.vector.tensor_mul(ynorm[64:128, ycol:ycol + 512],
                                 y1[0:64, :], rcp[64:128, :])

        def attn_qt(qt):
            nkb = (qt + 1) * 4
            for p in range(PAIRS):
                y0 = ypool.tile([65, 512], F32, tag="y0")
                y1 = ypool.tile([65, 512], F32, tag="y1")
                for kb in range(nkb):
                    o = kb - qt * 4
                    scol = max(0, o * 128)
                    width = 512 - scol
                    qcol = p * T + qt * 512 + scol
                    kcol = p * T + kb * 128
                    vbase = p * VSTRIDE + kb * 130
                    # both heads' scores in one 2-bank PSUM tile so a
                    # single ACT instruction exponentiates both
                    s01 = spool.tile([128, 1024], F32, tag="s01")
                    nc.tensor.matmul(
                        s01[:, 0:width],
                        lhsT=(kT[0:64, kcol:kcol + 128]),
                        rhs=(qT[0:64, qcol:qcol + width]),
                        start=True, stop=(o < 0))
                    nc.tensor.matmul(
                        s01[:, 512:512 + width],
                        lhsT=(kT[64:128, kcol:kcol + 128]),
                        rhs=(qT[64:128, qcol:qcol + width]),
                        start=True, stop=(o < 0))
                    if o >= 0:
                        # causal mask: accumulate -38 into masked positions
                        # of the diagonal 128-col chunk (both heads in one
                        # N=256 matmul: fp32r needs N>=256 for full rate)
                        mview = s01[:].rearrange("r (h x) -> r h x",
                                                 h=2)[:, :, 0:128]
                        nc.tensor.matmul(
                            mview, lhsT=ident_sb[:],
                            rhs=mask2_sb[:].rearrange("r (h x) -> r h x",
                                                      h=2),
                            start=False, stop=True)
                    p01 = ppool.tile([128, 1024], BF16, tag="p01")
                    sview = s01[:].rearrange("r (h x) -> r h x",
                                             h=2)[:, :, 0:width]
                    pview = p01[:].rearrange("r (h x) -> r h x",
                                             h=2)[:, :, 0:width]
                    nc.scalar.activation(
                        pview, sview, mybir.ActivationFunctionType.Exp)
                    nc.tensor.matmul(
                        y0[:, scol:512],
                        lhsT=(vnat[:, vbase:vbase + 65]),
                        rhs=(p01[:, 0:width]),
                        start=(kb == 0), stop=(kb == nkb - 1))
                    nc.tensor.matmul(
                        y1[:, scol:512],
                        lhsT=(vnat[:, vbase + 65:vbase + 130]),
                        rhs=(p01[:, 512:512 + width]),
                        start=(kb == 0), stop=(kb == nkb - 1))
                    if kb == 1 and pending[0] is not None:
                        pending[0]()
                        pending[0] = None
                if pending[0] is not None:
                    pending[0]()
                pending[0] = (lambda p=p, qt=qt, y0=y0, y1=y1:
                              _normalize(p, qt, y0, y1))

        def oproj_qt(qt):
            # flush the last pair's normalize so ynorm for qt is complete
            if pending[0] is not None:
                pending[0]()
                pending[0] = None
            for tt in range(qt * 4, qt * 4 + 4):
                osb = opool.tile([128, 1024], F32, tag="osb")
                for n in range(2):
                    ps = shpool.tile([128, 512], F32, tag="ps")
                    for p in range(PAIRS):
                        nc.tensor.matmul(
                            ps[:],
                            lhsT=(ynorm[:, p * T + tt * 128:
                                          p * T + tt * 128 + 128]),
                            rhs=(wo_sb[p][:, n * 512:(n + 1) * 512]),
                            start=(p == 0), stop=(p == PAIRS - 1))
                    nc.vector.tensor_copy(osb[:, n * 512:(n + 1) * 512],
                                          ps[:])
                nc.sync.dma_start(out[tt * 128:(tt + 1) * 128, :], osb[:])

        # ---------------- emission order -----------------------------
        prologue_dmas()
        proj_chunk(0, 0)
        attn_qt(0)
        proj_chunk(0, 1)
        oproj_qt(0)
        attn_qt(1)
        load_x(1)
        proj_chunk(1, 0)
        oproj_qt(1)
        attn_qt(2)
        proj_chunk(1, 1)
        oproj_qt(2)
        attn_qt(3)
        oproj_qt(3)


def shard_inputs(x, Wq, Wk, Wv, Wo):
    """Returns in_maps for cores 0..7 (core c: batch c//2, group c%2)."""
    import ml_dtypes
    x = np.ascontiguousarray(np.asarray(x, np.float32))
    mask1 = np.zeros((128, 128), np.float32)
    for r in range(128):
        mask1[r, :r] = -38.0  # S^T[key r, query j]: masked iff j < r
    mask = np.ascontiguousarray(np.concatenate([mask1, mask1], axis=1))
    ident = np.eye(128, dtype=np.float32)
    sel0 = np.zeros((1, 128), np.float32); sel0[0, :64] = 1.0
    sel1 = np.zeros((1, 128), np.float32); sel1[0, 64:] = 1.0
    in_maps = []
    perms = []
    for g in range(GROUPS):
        perm = np.array([(g * HPG + 2 * p + (q >= 64)) * 64 + (q % 64)
                         for p in range(PAIRS) for q in range(128)])
        perms.append(perm)
    w_cache = {}
    qscale = 1.0 / np.sqrt(DH)
    for g in range(GROUPS):
        perm = perms[g]
        wqT = (np.asarray(Wq, np.float32).T * qscale)[:, perm]
        wkT = np.asarray(Wk, np.float32).T[:, perm]
        # [r, c*512 + p*128 + o] = wT[c*128 + r, p*128 + o]
        def _re(wT):
            w4 = wT.reshape(8, 128, 4, 128)        # [c, r, p, o]
            return np.ascontiguousarray(
                w4.transpose(1, 0, 2, 3).reshape(128, 4096))
        w_cache[g] = {
            "wqR": _re(wqT),
            "wkR": _re(wkT),
            "wvT": np.ascontiguousarray(np.asarray(Wv, np.float32).T[:, perm]),
            "woT": np.ascontiguousarray(
                np.asarray(Wo, np.float32).T[perm, :]).astype(
                    ml_dtypes.bfloat16),
        }
    for c in range(N_CORES):
        b, g = c // 2, c % 2
        in_maps.append({
            "xT": np.ascontiguousarray(x[b].T),
            "maskR": mask, "identR": ident,
            "onesb": np.ones((128, 128), ml_dtypes.bfloat16),
            "sel0": sel0, "sel1": sel1,
            **w_cache[g],
        })
    return in_maps


def kernel(x, Wq, Wk, Wv, Wo):
    nc = build_nc()
    in_maps = shard_inputs(x, Wq, Wk, Wv, Wo)
    res = run_bass_kernel_spmd(nc, in_maps, list(range(N_CORES)))
    out = np.empty((B, T, D), np.float32)
    for b in range(B):
        out[b] = res.results[2 * b]["out"] + res.results[2 * b + 1]["out"]
    return out


# revision 14
# speedup vs baseline: 1.4163x; 1.1091x over previous
"""Causal self-attention (B=4, T=2048, D=1024, H=16) on 8 TRN2 NeuronCores.

Sharding: 2D (batch x head-group). Core c handles batch b = c//2 and head
group g = c%2 (8 heads, processed as 4 pairs).

v2 layout strategy (per core):
  - x is passed pre-transposed from host: xT [D, T].
  - Q/K projections produce qT/kT [128 local dims, T] with head pair 2p/2p+1
    stacked on partitions 0-63 / 64-127; the 1/sqrt(dh) scale is folded into
    the Wq weights on the host.
  - V is projected directly into natural [token, dim] layout (lhsT = x chunk,
    rhs = WvT) -- no PE transposes -- stored bf16 with a ones column per head
    so the PV matmul also accumulates the softmax denominator.
  - Scores are computed transposed: S^T [keys, queries]; causal masking is an
    accumulating PE matmul that adds -38 to masked positions BEFORE exp
    (start/stop accumulation group), so no vector mask multiply is needed.
  - exp() runs without max-subtraction (scores ~N(0,1), fp32 exp safe);
    output P^T is bf16, feeding the PV matmul directly.
  - Softmax denominators: row 64 of the y PSUM tiles -> 1/d via the fast
    custom-DVE reciprocal (reciprocal_approx_fast), broadcast to 64
    partitions with two col-tiled K=1 matmuls, one multiply per head.
  - o_proj consumes ynorm (bf16) as stationary; Wo is bf16; each core emits
    a partial [T, D] product over its 512 local head dims; host sums the two
    partials per batch.

Emission order interleaves projection / attention / o_proj so the Tile
scheduler can fill TensorE gaps during ACT-bound attention stretches and
keep the PE HAM-warm.
"""

import os
import sys

import numpy as np

if not any(os.path.isdir(os.path.join(p, "concourse")) for p in sys.path):
    sys.path.insert(0, "/opt/trn_rl_repo")

import concourse.mybir as mybir
import concourse.tile as tile
from concourse import bacc
from concourse.bass_utils import run_bass_kernel_spmd

B, T, D, H, DH = 4, 2048, 1024, 16, 64
N_CORES = 8
GROUPS = 2          # head groups (tensor-parallel dim)
HPG = H // GROUPS   # heads per group/core
PAIRS = HPG // 2    # head pairs per core
NKB = T // 128      # 128-key blocks per batch
NQT = T // 512      # 512-query tiles per batch
VSTRIDE = NKB * 130 # vnat cols per pair: 16 blocks x [64 dims|1|64 dims|1]

F32 = mybir.dt.float32
F32R = mybir.dt.float32r
BF16 = mybir.dt.bfloat16


def build_nc():
    nc = bacc.Bacc("TRN2", target_bir_lowering=False, debug=False,
                   num_devices=N_CORES)
    xT = nc.dram_tensor("xT", [D, T], BF16, kind="ExternalInput").ap()
    wqR = nc.dram_tensor("wqR", [128, 4096], BF16, kind="ExternalInput").ap()
    wkR = nc.dram_tensor("wkR", [128, 4096], BF16, kind="ExternalInput").ap()
    wvT = nc.dram_tensor("wvT", [D, 512], BF16, kind="ExternalInput").ap()
    woT = nc.dram_tensor("woT", [512, D], BF16, kind="ExternalInput").ap()
    maskR = nc.dram_tensor("maskR", [128, 256], BF16, kind="ExternalInput").ap()
    identR = nc.dram_tensor("identR", [128, 128], BF16, kind="ExternalInput").ap()
    onesb = nc.dram_tensor("onesb", [128, 128], BF16, kind="ExternalInput").ap()
    sel0 = nc.dram_tensor("sel0", [1, 128], F32R, kind="ExternalInput").ap()
    sel1 = nc.dram_tensor("sel1", [1, 128], F32R, kind="ExternalInput").ap()
    out = nc.dram_tensor("out", [T, D], F32, kind="ExternalOutput").ap()

    with tile.TileContext(nc) as tc:
        _body(tc, out, xT, wqR, wkR, wvT, woT, maskR, identR, onesb, sel0, sel1)
    nc.compile()
    return nc


def _body(tc, out, xT, wqR, wkR, wvT, woT, maskR, identR, onesb, sel0, sel1):
    nc = tc.nc
    from contextlib import ExitStack

    with ExitStack() as ctx:
        persist = ctx.enter_context(tc.tile_pool(name="persist", bufs=1))
        qT = persist.tile([128, PAIRS * T], BF16, tag="qT")
        kT = persist.tile([128, PAIRS * T], BF16, tag="kT")
        vnat = persist.tile([128, PAIRS * VSTRIDE], BF16, tag="vnat")
        ynorm = persist.tile([128, PAIRS * T], BF16, tag="ynorm")

        consts = ctx.enter_context(tc.tile_pool(name="consts", bufs=1))
        mask2_sb = consts.tile([128, 256], BF16, tag="mask")
        nc.sync.dma_start(mask2_sb[:], maskR[:])
        ident_sb = consts.tile([128, 128], BF16, tag="ident")
        nc.sync.dma_start(ident_sb[:], identR[:])
        sel0_sb = consts.tile([1, 128], F32R, tag="sel0")
        nc.sync.dma_start(sel0_sb[:], sel0[:])
        sel1_sb = consts.tile([1, 128], F32R, tag="sel1")
        nc.sync.dma_start(sel1_sb[:], sel1[:])

        # ones columns of vnat (denominator accumulators): col 64 of each
        # 65-wide [dims|1] half-chunk, DMA'd from a DRAM bf16 constant.
        ones_view = vnat[:].rearrange("r (p k m x) -> r (p k m) x",
                                      p=PAIRS, k=NKB, m=2)[:, :, 64:65]
        nc.sync.dma_start(ones_view.squeeze(), onesb[:])

        wqkpool = ctx.enter_context(tc.tile_pool(name="wqk", bufs=1))
        wq_sb = wqkpool.tile([128, 4096], BF16, tag="wq")
        wk_sb = wqkpool.tile([128, 4096], BF16, tag="wk")
        wvpool = ctx.enter_context(tc.tile_pool(name="wv", bufs=1))
        wv_sb = wvpool.tile([128, 8 * 512], BF16, tag="wv")
        wopool = ctx.enter_context(tc.tile_pool(name="wo", bufs=1))
        wo_sb = []
        for p in range(PAIRS):
            wot = wopool.tile([128, 1024], BF16, tag=f"wo{p}")
            wo_sb.append(wot)

        xpool = ctx.enter_context(tc.tile_pool(name="xt", bufs=2))
        ppool = ctx.enter_context(tc.tile_pool(name="p", bufs=4))
        rpool = ctx.enter_context(tc.tile_pool(name="r", bufs=2))
        opool = ctx.enter_context(tc.tile_pool(name="osb", bufs=3))

        spool = ctx.enter_context(
            tc.tile_pool(name="s", bufs=2, space="PSUM"))
        ypool = ctx.enter_context(
            tc.tile_pool(name="y", bufs=1, space="PSUM"))
        shpool = ctx.enter_context(
            tc.tile_pool(name="sh", bufs=2, space="PSUM"))

        # ---------------- phase emitters -----------------------------
        x_sb = [None] * 8

        def load_x(half):
            for c in range(8):
                xt = xpool.tile([128, 1024], BF16, tag=f"x{c}")
                nc.sync.dma_start(
                    xt[:], xT[c * 128:(c + 1) * 128,
                              half * 1024:(half + 1) * 1024])
                x_sb[c] = xt

        def prologue_dmas():
            # ordered so the first q-proj matmul can start after ~1MB:
            # wq strip c + x strip c interleaved, then wk, wv, wo
            for c in range(8):
                nc.sync.dma_start(wq_sb[:, c * 512:(c + 1) * 512],
                                  wqR[:, c * 512:(c + 1) * 512])
                xt = xpool.tile([128, 1024], BF16, tag=f"x{c}")
                nc.sync.dma_start(xt[:], xT[c * 128:(c + 1) * 128, 0:1024])
                x_sb[c] = xt
            for c in range(8):
                nc.sync.dma_start(wk_sb[:, c * 512:(c + 1) * 512],
                                  wkR[:, c * 512:(c + 1) * 512])
            for c in range(8):
                nc.sync.dma_start(wv_sb[:, c * 512:(c + 1) * 512],
                                  wvT[c * 128:(c + 1) * 128, :])
            for p in range(PAIRS):
                nc.sync.dma_start(wo_sb[p][:], woT[p * 128:(p + 1) * 128, :])

        def proj_chunk(half, sub):
            # q/k for 512 tokens
            for w_sb, dst in ((wq_sb, qT), (wk_sb, kT)):
                for p in range(PAIRS):
                    ps = shpool.tile([128, 512], F32, tag="ps")
                    for c in range(8):
                        nc.tensor.matmul(
                            ps[:],
                            lhsT=(w_sb[:, c * 512 + p * 128:
                                         c * 512 + (p + 1) * 128]),
                            rhs=(x_sb[c][:, sub * 512:(sub + 1) * 512]),
                            start=(c == 0), stop=(c == 7))
                    col0 = p * T + half * 1024 + sub * 512
                    nc.vector.tensor_copy(dst[:, col0:col0 + 512], ps[:])
            # V in natural [token, dim] layout: 128-token blocks
            for tb in range(4):
                ps = shpool.tile([128, 512], F32, tag="ps")
                tok0 = sub * 512 + tb * 128
                for c in range(8):
                    nc.tensor.matmul(
                        ps[:],
                        lhsT=(x_sb[c][:, tok0:tok0 + 128]),
                        rhs=(wv_sb[:, c * 512:(c + 1) * 512]),
                        start=(c == 0), stop=(c == 7))
                kb = half * 8 + sub * 4 + tb
                srcv = ps[:].rearrange("r (p m x) -> r p m x", p=PAIRS, m=2)
                dstv = vnat[:].rearrange(
                    "r (p k m x) -> r p k m x",
                    p=PAIRS, k=NKB, m=2)[:, :, kb:kb + 1, :, 0:64]
                nc.vector.tensor_copy(dstv.squeeze(2), srcv)

        pending = [None]

        def _normalize(p, qt, y0, y1):
            den0 = rpool.tile([1, 512], F32R, tag="den0")
            den1 = rpool.tile([1, 512], F32R, tag="den1")
            nc.vector.tensor_copy(den0[:], y0[64:65, :])
            nc.vector.tensor_copy(den1[:], y1[64:65, :])
            # broadcast raw denominators to 64 partitions each (col-tiled
            # K=1 matmuls), then one fast-approx reciprocal on the full
            # [128, 512] tile (DVE cost is free-dim-bound, so this is as
            # cheap as a [1, 512] reciprocal)
            rbs = shpool.tile([128, 512], F32, tag="ps")
            nc.tensor.matmul(rbs[:], lhsT=sel0_sb[:],
                             rhs=den0[:], start=True, stop=False)
            nc.tensor.matmul(rbs[:], lhsT=sel1_sb[:],
                             rhs=den1[:], start=False, stop=True)
            rcp = rpool.tile([128, 512], F32, tag="rcp")
            nc.vector.reciprocal_approx_fast(out=rcp[:], in_=rbs[:])
            ycol = p * T + qt * 512
            nc.vector.tensor_mul(ynorm[0:64, ycol:ycol + 512],
                                 y0[0:64, :], rcp[0:64, :])
            nc.vector.tensor_mul(ynorm[64:128, ycol:ycol + 512],
                                 y1[0:64, :], rcp[64:128, :])

        def attn_qt(qt):
            nkb = (qt + 1) * 4
            for p in range(PAIRS):
                y0 = ypool.tile([65, 512], F32, tag="y0")
                y1 = ypool.tile([65, 512], F32, tag="y1")
                for kb in range(nkb):
                    o = kb - qt * 4
                    scol = max(0, o * 128)
                    width = 512 - scol
                    qcol = p * T + qt * 512 + scol
                    kcol = p * T + kb * 128
                    vbase = p * VSTRIDE + kb * 130
                    # both heads' scores in one 2-bank PSUM tile so a
                    # single ACT instruction exponentiates both
                    s01 = spool.tile([128, 1024], F32, tag="s01")
                    nc.tensor.matmul(
                        s01[:, 0:width],
                        lhsT=(kT[0:64, kcol:kcol + 128]),
                        rhs=(qT[0:64, qcol:qcol + width]),
                        start=True, stop=(o < 0))
                    nc.tensor.matmul(
                        s01[:, 512:512 + width],
                        lhsT=(kT[64:128, kcol:kcol + 128]),
                        rhs=(qT[64:128, qcol:qcol + width]),
                        start=True, stop=(o < 0))
                    if o >= 0:
                        # causal mask: accumulate -38 into masked positions
                        # of the diagonal 128-col chunk (both heads in one
                        # N=256 matmul: fp32r needs N>=256 for full rate)
                        mview = s01[:].rearrange("r (h x) -> r h x",
                                                 h=2)[:, :, 0:128]
                        nc.tensor.matmul(
                            mview, lhsT=ident_sb[:],
                            rhs=mask2_sb[:].rearrange("r (h x) -> r h x",
                                                      h=2),
                            start=False, stop=True)
                    p01 = ppool.tile([128, 1024], BF16, tag="p01")
                    sview = s01[:].rearrange("r (h x) -> r h x",
                                             h=2)[:, :, 0:width]
                    pview = p01[:].rearrange("r (h x) -> r h x",
                                             h=2)[:, :, 0:width]
                    nc.scalar.activation(
                        pview, sview, mybir.ActivationFunctionType.Exp)
                    nc.tensor.matmul(
                        y0[:, scol:512],
                        lhsT=(vnat[:, vbase:vbase + 65]),
                        rhs=(p01[:, 0:width]),
                        start=(kb == 0), stop=(kb == nkb - 1))
                    nc.tensor.matmul(
                        y1[:, scol:512],
                        lhsT=(vnat[:, vbase + 65:vbase + 130]),
                        rhs=(p01[:, 512:512 + width]),
                        start=(kb == 0), stop=(kb == nkb - 1))
                    if kb == 1 and pending[0] is not None:
                        pending[0]()
                        pending[0] = None
                if pending[0] is not None:
                    pending[0]()
                pending[0] = (lambda p=p, qt=qt, y0=y0, y1=y1:
                              _normalize(p, qt, y0, y1))

        def oproj_qt(qt):
            # flush the last pair's normalize so ynorm for qt is complete
            if pending[0] is not None:
                pending[0]()
                pending[0] = None
            for tt in range(qt * 4, qt * 4 + 4):
                osb = opool.tile([128, 1024], F32, tag="osb")
                for n in range(2):
                    ps = shpool.tile([128, 512], F32, tag="ps")
                    for p in range(PAIRS):
                        nc.tensor.matmul(
                            ps[:],
                            lhsT=(ynorm[:, p * T + tt * 128:
                                          p * T + tt * 128 + 128]),
                            rhs=(wo_sb[p][:, n * 512:(n + 1) * 512]),
                            start=(p == 0), stop=(p == PAIRS - 1))
                    nc.vector.tensor_copy(osb[:, n * 512:(n + 1) * 512],
                                          ps[:])
                nc.sync.dma_start(out[tt * 128:(tt + 1) * 128, :], osb[:])

        # ---------------- emission order -----------------------------
        prologue_dmas()
        proj_chunk(0, 0)
        attn_qt(0)
        proj_chunk(0, 1)
        oproj_qt(0)
        attn_qt(1)
        load_x(1)
        proj_chunk(1, 0)
        oproj_qt(1)
        attn_qt(2)
        proj_chunk(1, 1)
        oproj_qt(2)
        attn_qt(3)
        oproj_qt(3)


def shard_inputs(x, Wq, Wk, Wv, Wo):
    """Returns in_maps for cores 0..7 (core c: batch c//2, group c%2)."""
    import ml_dtypes
    x = np.ascontiguousarray(np.asarray(x, np.float32))
    mask1 = np.zeros((128, 128), np.float32)
    for r in range(128):
        mask1[r, :r] = -38.0  # S^T[key r, query j]: masked iff j < r
    mask = np.ascontiguousarray(np.concatenate([mask1, mask1], axis=1))
    ident = np.eye(128, dtype=np.float32)
    sel0 = np.zeros((1, 128), np.float32); sel0[0, :64] = 1.0
    sel1 = np.zeros((1, 128), np.float32); sel1[0, 64:] = 1.0
    in_maps = []
    perms = []
    for g in range(GROUPS):
        perm = np.array([(g * HPG + 2 * p + (q >= 64)) * 64 + (q % 64)
                         for p in range(PAIRS) for q in range(128)])
        perms.append(perm)
    w_cache = {}
    qscale = 1.0 / np.sqrt(DH)
    for g in range(GROUPS):
        perm = perms[g]
        wqT = (np.asarray(Wq, np.float32).T * qscale)[:, perm]
        wkT = np.asarray(Wk, np.float32).T[:, perm]
        # [r, c*512 + p*128 + o] = wT[c*128 + r, p*128 + o]
        def _re(wT):
            w4 = wT.reshape(8, 128, 4, 128)        # [c, r, p, o]
            return np.ascontiguousarray(
                w4.transpose(1, 0, 2, 3).reshape(128, 4096))
        w_cache[g] = {
            "wqR": _re(wqT).astype(ml_dtypes.bfloat16),
            "wkR": _re(wkT).astype(ml_dtypes.bfloat16),
            "wvT": np.ascontiguousarray(
                np.asarray(Wv, np.float32).T[:, perm]).astype(
                    ml_dtypes.bfloat16),
            "woT": np.ascontiguousarray(
                np.asarray(Wo, np.float32).T[perm, :]).astype(
                    ml_dtypes.bfloat16),
        }
    for c in range(N_CORES):
        b, g = c // 2, c % 2
        in_maps.append({
            "xT": np.ascontiguousarray(x[b].T).astype(ml_dtypes.bfloat16),
            "maskR": mask.astype(ml_dtypes.bfloat16),
            "identR": ident.astype(ml_dtypes.bfloat16),
            "onesb": np.ones((128, 128), ml_dtypes.bfloat16),
            "sel0": sel0, "sel1": sel1,
            **w_cache[g],
        })
    return in_maps


def kernel(x, Wq, Wk, Wv, Wo):
    nc = build_nc()
    in_maps = shard_inputs(x, Wq, Wk, Wv, Wo)
    res = run_bass_kernel_spmd(nc, in_maps, list(range(N_CORES)))
    out = np.empty((B, T, D), np.float32)
    for b in range(B):
        out[b] = res.results[2 * b]["out"] + res.results[2 * b + 1]["out"]
    return out


# revision 16
# speedup vs baseline: 1.4430x; 1.0189x over previous
"""Causal self-attention (B=4, T=2048, D=1024, H=16) on 8 TRN2 NeuronCores.

Sharding: 2D (batch x head-group). Core c handles batch b = c//2 and head
group g = c%2 (8 heads, processed as 4 pairs).

v2 layout strategy (per core):
  - x is passed pre-transposed from host: xT [D, T].
  - Q/K projections produce qT/kT [128 local dims, T] with head pair 2p/2p+1
    stacked on partitions 0-63 / 64-127; the 1/sqrt(dh) scale is folded into
    the Wq weights on the host.
  - V is projected directly into natural [token, dim] layout (lhsT = x chunk,
    rhs = WvT) -- no PE transposes -- stored bf16 with a ones column per head
    so the PV matmul also accumulates the softmax denominator.
  - Scores are computed transposed: S^T [keys, queries]; causal masking is an
    accumulating PE matmul that adds -38 to masked positions BEFORE exp
    (start/stop accumulation group), so no vector mask multiply is needed.
  - exp() runs without max-subtraction (scores ~N(0,1), fp32 exp safe);
    output P^T is bf16, feeding the PV matmul directly.
  - Softmax denominators: row 64 of the y PSUM tiles -> 1/d via the fast
    custom-DVE reciprocal (reciprocal_approx_fast), broadcast to 64
    partitions with two col-tiled K=1 matmuls, one multiply per head.
  - o_proj consumes ynorm (bf16) as stationary; Wo is bf16; each core emits
    a partial [T, D] product over its 512 local head dims; host sums the two
    partials per batch.

Emission order interleaves projection / attention / o_proj so the Tile
scheduler can fill TensorE gaps during ACT-bound attention stretches and
keep the PE HAM-warm.
"""

import os
import sys

import numpy as np

if not any(os.path.isdir(os.path.join(p, "concourse")) for p in sys.path):
    sys.path.insert(0, "/opt/trn_rl_repo")

import concourse.mybir as mybir
import concourse.tile as tile
from concourse import bacc
from concourse.bass_utils import run_bass_kernel_spmd

B, T, D, H, DH = 4, 2048, 1024, 16, 64
N_CORES = 8
GROUPS = 2          # head groups (tensor-parallel dim)
HPG = H // GROUPS   # heads per group/core
PAIRS = HPG // 2    # head pairs per core
NKB = T // 128      # 128-key blocks per batch
NQT = T // 512      # 512-query tiles per batch
VSTRIDE = NKB * 130 # vnat cols per pair: 16 blocks x [64 dims|1|64 dims|1]

F32 = mybir.dt.float32
F32R = mybir.dt.float32r
BF16 = mybir.dt.bfloat16


def build_nc():
    nc = bacc.Bacc("TRN2", target_bir_lowering=False, debug=False,
                   num_devices=N_CORES)
    xT = nc.dram_tensor("xT", [D, T], BF16, kind="ExternalInput").ap()
    wqR = nc.dram_tensor("wqR", [128, 4096], BF16, kind="ExternalInput").ap()
    wkR = nc.dram_tensor("wkR", [128, 4096], BF16, kind="ExternalInput").ap()
    wvT = nc.dram_tensor("wvT", [D, 512], BF16, kind="ExternalInput").ap()
    woT = nc.dram_tensor("woT", [512, D], BF16, kind="ExternalInput").ap()
    cpk = nc.dram_tensor("cpk", [128, 384], BF16, kind="ExternalInput").ap()
    onesb = nc.dram_tensor("onesb", [128, 128], BF16, kind="ExternalInput").ap()
    sel0 = nc.dram_tensor("sel0", [1, 128], F32R, kind="ExternalInput").ap()
    sel1 = nc.dram_tensor("sel1", [1, 128], F32R, kind="ExternalInput").ap()
    out = nc.dram_tensor("out", [T, D], F32, kind="ExternalOutput").ap()

    with tile.TileContext(nc) as tc:
        _body(tc, out, xT, wqR, wkR, wvT, woT, cpk, onesb, sel0, sel1)
    nc.compile()
    return nc


def _body(tc, out, xT, wqR, wkR, wvT, woT, cpk, onesb, sel0, sel1):
    nc = tc.nc
    from contextlib import ExitStack

    with ExitStack() as ctx:
        persist = ctx.enter_context(tc.tile_pool(name="persist", bufs=1))
        qT = persist.tile([128, PAIRS * T], BF16, tag="qT")
        kT = persist.tile([128, PAIRS * T], BF16, tag="kT")
        vnat = persist.tile([128, PAIRS * VSTRIDE], BF16, tag="vnat")
        ynorm = persist.tile([128, PAIRS * T], BF16, tag="ynorm")

        consts = ctx.enter_context(tc.tile_pool(name="consts", bufs=1))
        cpk_sb = consts.tile([128, 384], BF16, tag="cpk")
        nc.sync.dma_start(cpk_sb[:], cpk[:])
        mask2_sb = cpk_sb[:, 0:256]
        ident_sb = cpk_sb[:, 256:384]
        sel0_sb = consts.tile([1, 128], F32R, tag="sel0")
        sel1_sb = consts.tile([1, 128], F32R, tag="sel1")
        warm = consts.tile([128, 512], BF16, tag="warm")
        nc.vector.memset(warm[:], 0.0)

        wqkpool = ctx.enter_context(tc.tile_pool(name="wqk", bufs=1))
        wq_sb = wqkpool.tile([128, 4096], BF16, tag="wq")
        wk_sb = wqkpool.tile([128, 4096], BF16, tag="wk")
        wvpool = ctx.enter_context(tc.tile_pool(name="wv", bufs=1))
        wv_sb = wvpool.tile([128, 8 * 512], BF16, tag="wv")
        wopool = ctx.enter_context(tc.tile_pool(name="wo", bufs=1))
        wo_sb = []
        for p in range(PAIRS):
            wot = wopool.tile([128, 1024], BF16, tag=f"wo{p}")
            wo_sb.append(wot)

        xpool = ctx.enter_context(tc.tile_pool(name="xt", bufs=2))
        ppool = ctx.enter_context(tc.tile_pool(name="p", bufs=4))
        rpool = ctx.enter_context(tc.tile_pool(name="r", bufs=2))
        opool = ctx.enter_context(tc.tile_pool(name="osb", bufs=3))

        spool = ctx.enter_context(
            tc.tile_pool(name="s", bufs=2, space="PSUM"))
        ypool = ctx.enter_context(
            tc.tile_pool(name="y", bufs=1, space="PSUM"))
        shpool = ctx.enter_context(
            tc.tile_pool(name="sh", bufs=2, space="PSUM"))

        # ---------------- phase emitters -----------------------------
        x_sb = [None] * 8

        def load_x(half):
            for c in range(8):
                xt = xpool.tile([128, 1024], BF16, tag=f"x{c}")
                nc.sync.dma_start(
                    xt[:], xT[c * 128:(c + 1) * 128,
                              half * 1024:(half + 1) * 1024])
                x_sb[c] = xt

        def prologue_dmas():
            # warm-up burst: dummy matmuls while DMAs stream, so the PE
            # HAM un-throttles before the first real projection matmul
            wps = shpool.tile([128, 512], F32, tag="ps")
            for i in range(14):
                nc.tensor.matmul(wps[:], lhsT=warm[:, 0:128], rhs=warm[:],
                                 start=True, stop=True)
            # wq strips on sync queue, x strips on gpsimd queue: parallel
            # issue so the first q-proj matmul can start after ~0.8MB
            for c in range(8):
                nc.sync.dma_start(wq_sb[:, c * 512:(c + 1) * 512],
                                  wqR[:, c * 512:(c + 1) * 512])
                xt = xpool.tile([128, 1024], BF16, tag=f"x{c}")
                nc.gpsimd.dma_start(xt[:], xT[c * 128:(c + 1) * 128, 0:1024])
                x_sb[c] = xt
            for c in range(8):
                nc.sync.dma_start(wk_sb[:, c * 512:(c + 1) * 512],
                                  wkR[:, c * 512:(c + 1) * 512])
            ones_view = vnat[:].rearrange("r (p k m x) -> r (p k m) x",
                                          p=PAIRS, k=NKB, m=2)[:, :, 64:65]
            nc.sync.dma_start(ones_view.squeeze(), onesb[:])
            for c in range(8):
                nc.sync.dma_start(wv_sb[:, c * 512:(c + 1) * 512],
                                  wvT[c * 128:(c + 1) * 128, :])
            for p in range(PAIRS):
                nc.sync.dma_start(wo_sb[p][:], woT[p * 128:(p + 1) * 128, :])
            nc.gpsimd.dma_start(sel0_sb[:], sel0[:])
            nc.gpsimd.dma_start(sel1_sb[:], sel1[:])

        def proj_chunk(half, sub):
            # q/k for 512 tokens
            for w_sb, dst in ((wq_sb, qT), (wk_sb, kT)):
                for p in range(PAIRS):
                    ps = shpool.tile([128, 512], F32, tag="ps")
                    for c in range(8):
                        nc.tensor.matmul(
                            ps[:],
                            lhsT=(w_sb[:, c * 512 + p * 128:
                                         c * 512 + (p + 1) * 128]),
                            rhs=(x_sb[c][:, sub * 512:(sub + 1) * 512]),
                            start=(c == 0), stop=(c == 7))
                    col0 = p * T + half * 1024 + sub * 512
                    nc.vector.tensor_copy(dst[:, col0:col0 + 512], ps[:])
            # V in natural [token, dim] layout: 128-token blocks
            for tb in range(4):
                ps = shpool.tile([128, 512], F32, tag="ps")
                tok0 = sub * 512 + tb * 128
                for c in range(8):
                    nc.tensor.matmul(
                        ps[:],
                        lhsT=(x_sb[c][:, tok0:tok0 + 128]),
                        rhs=(wv_sb[:, c * 512:(c + 1) * 512]),
                        start=(c == 0), stop=(c == 7))
                kb = half * 8 + sub * 4 + tb
                srcv = ps[:].rearrange("r (p m x) -> r p m x", p=PAIRS, m=2)
                dstv = vnat[:].rearrange(
                    "r (p k m x) -> r p k m x",
                    p=PAIRS, k=NKB, m=2)[:, :, kb:kb + 1, :, 0:64]
                nc.vector.tensor_copy(dstv.squeeze(2), srcv)

        pending = [None]

        def _normalize(p, qt, y0, y1):
            den0 = rpool.tile([1, 512], F32R, tag="den0")
            den1 = rpool.tile([1, 512], F32R, tag="den1")
            nc.vector.tensor_copy(den0[:], y0[64:65, :])
            nc.vector.tensor_copy(den1[:], y1[64:65, :])
            # broadcast raw denominators to 64 partitions each (col-tiled
            # K=1 matmuls), then one fast-approx reciprocal on the full
            # [128, 512] tile (DVE cost is free-dim-bound, so this is as
            # cheap as a [1, 512] reciprocal)
            rbs = shpool.tile([128, 512], F32, tag="ps")
            nc.tensor.matmul(rbs[:], lhsT=sel0_sb[:],
                             rhs=den0[:], start=True, stop=False)
            nc.tensor.matmul(rbs[:], lhsT=sel1_sb[:],
                             rhs=den1[:], start=False, stop=True)
            rcp = rpool.tile([128, 512], F32, tag="rcp")
            nc.vector.reciprocal_approx_fast(out=rcp[:], in_=rbs[:])
            ycol = p * T + qt * 512
            nc.vector.tensor_mul(ynorm[0:64, ycol:ycol + 512],
                                 y0[0:64, :], rcp[0:64, :])
            nc.vector.tensor_mul(ynorm[64:128, ycol:ycol + 512],
                                 y1[0:64, :], rcp[64:128, :])

        def attn_qt(qt):
            nkb = (qt + 1) * 4
            for p in range(PAIRS):
                y0 = ypool.tile([65, 512], F32, tag="y0")
                y1 = ypool.tile([65, 512], F32, tag="y1")
                for kb in range(nkb):
                    o = kb - qt * 4
                    scol = max(0, o * 128)
                    width = 512 - scol
                    qcol = p * T + qt * 512 + scol
                    kcol = p * T + kb * 128
                    vbase = p * VSTRIDE + kb * 130
                    # both heads' scores in one 2-bank PSUM tile so a
                    # single ACT instruction exponentiates both
                    s01 = spool.tile([128, 1024], F32, tag="s01")
                    nc.tensor.matmul(
                        s01[:, 0:width],
                        lhsT=(kT[0:64, kcol:kcol + 128]),
                        rhs=(qT[0:64, qcol:qcol + width]),
                        start=True, stop=(o < 0))
                    nc.tensor.matmul(
                        s01[:, 512:512 + width],
                        lhsT=(kT[64:128, kcol:kcol + 128]),
                        rhs=(qT[64:128, qcol:qcol + width]),
                        start=True, stop=(o < 0))
                    if o >= 0:
                        # causal mask: accumulate -38 into masked positions
                        # of the diagonal 128-col chunk (both heads in one
                        # N=256 matmul: fp32r needs N>=256 for full rate)
                        mview = s01[:].rearrange("r (h x) -> r h x",
                                                 h=2)[:, :, 0:128]
                        nc.tensor.matmul(
                            mview, lhsT=ident_sb,
                            rhs=mask2_sb.rearrange("r (h x) -> r h x",
                                                      h=2),
                            start=False, stop=True)
                    p01 = ppool.tile([128, 1024], BF16, tag="p01")
                    sview = s01[:].rearrange("r (h x) -> r h x",
                                             h=2)[:, :, 0:width]
                    pview = p01[:].rearrange("r (h x) -> r h x",
                                             h=2)[:, :, 0:width]
                    nc.scalar.activation(
                        pview, sview, mybir.ActivationFunctionType.Exp)
                    nc.tensor.matmul(
                        y0[:, scol:512],
                        lhsT=(vnat[:, vbase:vbase + 65]),
                        rhs=(p01[:, 0:width]),
                        start=(kb == 0), stop=(kb == nkb - 1))
                    nc.tensor.matmul(
                        y1[:, scol:512],
                        lhsT=(vnat[:, vbase + 65:vbase + 130]),
                        rhs=(p01[:, 512:512 + width]),
                        start=(kb == 0), stop=(kb == nkb - 1))
                    if kb == 1 and pending[0] is not None:
                        pending[0]()
                        pending[0] = None
                if pending[0] is not None:
                    pending[0]()
                pending[0] = (lambda p=p, qt=qt, y0=y0, y1=y1:
                              _normalize(p, qt, y0, y1))

        def oproj_qt(qt):
            # flush the last pair's normalize so ynorm for qt is complete
            if pending[0] is not None:
                pending[0]()
                pending[0] = None
            for tt in range(qt * 4, qt * 4 + 4):
                osb = opool.tile([128, 1024], F32, tag="osb")
                for n in range(2):
                    ps = shpool.tile([128, 512], F32, tag="ps")
                    for p in range(PAIRS):
                        nc.tensor.matmul(
                            ps[:],
                            lhsT=(ynorm[:, p * T + tt * 128:
                                          p * T + tt * 128 + 128]),
                            rhs=(wo_sb[p][:, n * 512:(n + 1) * 512]),
                            start=(p == 0), stop=(p == PAIRS - 1))
                    nc.vector.tensor_copy(osb[:, n * 512:(n + 1) * 512],
                                          ps[:])
                nc.gpsimd.dma_start(out[tt * 128:(tt + 1) * 128, :], osb[:])

        # ---------------- emission order -----------------------------
        prologue_dmas()
        proj_chunk(0, 0)
        attn_qt(0)
        proj_chunk(0, 1)
        oproj_qt(0)
        attn_qt(1)
        load_x(1)
        proj_chunk(1, 0)
        oproj_qt(1)
        attn_qt(2)
        proj_chunk(1, 1)
        oproj_qt(2)
        attn_qt(3)
        oproj_qt(3)


def shard_inputs(x, Wq, Wk, Wv, Wo):
    """Returns in_maps for cores 0..7 (core c: batch c//2, group c%2)."""
    import ml_dtypes
    x = np.ascontiguousarray(np.asarray(x, np.float32))
    mask1 = np.zeros((128, 128), np.float32)
    for r in range(128):
        mask1[r, :r] = -38.0  # S^T[key r, query j]: masked iff j < r
    mask = np.ascontiguousarray(np.concatenate([mask1, mask1], axis=1))
    ident = np.eye(128, dtype=np.float32)
    sel0 = np.zeros((1, 128), np.float32); sel0[0, :64] = 1.0
    sel1 = np.zeros((1, 128), np.float32); sel1[0, 64:] = 1.0
    in_maps = []
    perms = []
    for g in range(GROUPS):
        perm = np.array([(g * HPG + 2 * p + (q >= 64)) * 64 + (q % 64)
                         for p in range(PAIRS) for q in range(128)])
        perms.append(perm)
    w_cache = {}
    qscale = 1.0 / np.sqrt(DH)
    for g in range(GROUPS):
        perm = perms[g]
        wqT = (np.asarray(Wq, np.float32).T * qscale)[:, perm]
        wkT = np.asarray(Wk, np.float32).T[:, perm]
        # [r, c*512 + p*128 + o] = wT[c*128 + r, p*128 + o]
        def _re(wT):
            w4 = wT.reshape(8, 128, 4, 128)        # [c, r, p, o]
            return np.ascontiguousarray(
                w4.transpose(1, 0, 2, 3).reshape(128, 4096))
        w_cache[g] = {
            "wqR": _re(wqT).astype(ml_dtypes.bfloat16),
            "wkR": _re(wkT).astype(ml_dtypes.bfloat16),
            "wvT": np.ascontiguousarray(
                np.asarray(Wv, np.float32).T[:, perm]).astype(
                    ml_dtypes.bfloat16),
            "woT": np.ascontiguousarray(
                np.asarray(Wo, np.float32).T[perm, :]).astype(
                    ml_dtypes.bfloat16),
        }
    for c in range(N_CORES):
        b, g = c // 2, c % 2
        in_maps.append({
            "xT": np.ascontiguousarray(x[b].T).astype(ml_dtypes.bfloat16),
            "cpk": np.ascontiguousarray(
                np.concatenate([mask, ident], axis=1)).astype(
                    ml_dtypes.bfloat16),
            "onesb": np.ones((128, 128), ml_dtypes.bfloat16),
            "sel0": sel0, "sel1": sel1,
            **w_cache[g],
        })
    return in_maps


def kernel(x, Wq, Wk, Wv, Wo):
    nc = build_nc()
    in_maps = shard_inputs(x, Wq, Wk, Wv, Wo)
    res = run_bass_kernel_spmd(nc, in_maps, list(range(N_CORES)))
    out = np.empty((B, T, D), np.float32)
    for b in range(B):
        out[b] = res.results[2 * b]["out"] + res.results[2 * b + 1]["out"]
    return out


# revision 17
# speedup vs baseline: 1.5596x; 1.0808x over previous
"""Causal self-attention (B=4, T=2048, D=1024, H=16) on 8 TRN2 NeuronCores.

Sharding: 2D (batch x head-group). Core c handles batch b = c//2 and head
group g = c%2 (8 heads, processed as 4 pairs).

v2 layout strategy (per core):
  - x is passed pre-transposed from host: xT [D, T].
  - Q/K projections produce qT/kT [128 local dims, T] with head pair 2p/2p+1
    stacked on partitions 0-63 / 64-127; the 1/sqrt(dh) scale is folded into
    the Wq weights on the host.
  - V is projected directly into natural [token, dim] layout (lhsT = x chunk,
    rhs = WvT) -- no PE transposes -- stored bf16 with a ones column per head
    so the PV matmul also accumulates the softmax denominator.
  - Scores are computed transposed: S^T [keys, queries]; causal masking is an
    accumulating PE matmul that adds -38 to masked positions BEFORE exp
    (start/stop accumulation group), so no vector mask multiply is needed.
  - exp() runs without max-subtraction (scores ~N(0,1), fp32 exp safe);
    output P^T is bf16, feeding the PV matmul directly.
  - Softmax denominators: row 64 of the y PSUM tiles -> 1/d via the fast
    custom-DVE reciprocal (reciprocal_approx_fast), broadcast to 64
    partitions with two col-tiled K=1 matmuls, one multiply per head.
  - o_proj consumes ynorm (bf16) as stationary; Wo is bf16; each core emits
    a partial [T, D] product over its 512 local head dims; host sums the two
    partials per batch.

Emission order interleaves projection / attention / o_proj so the Tile
scheduler can fill TensorE gaps during ACT-bound attention stretches and
keep the PE HAM-warm.
"""

import os
import sys

import numpy as np

if not any(os.path.isdir(os.path.join(p, "concourse")) for p in sys.path):
    sys.path.insert(0, "/opt/trn_rl_repo")

import concourse.mybir as mybir
import concourse.tile as tile
from concourse import bacc
from concourse.bass_utils import run_bass_kernel_spmd

B, T, D, H, DH = 4, 2048, 1024, 16, 64
N_CORES = 8
GROUPS = 2          # head groups (tensor-parallel dim)
HPG = H // GROUPS   # heads per group/core
PAIRS = HPG // 2    # head pairs per core
NKB = T // 128      # 128-key blocks per batch
NQT = T // 512      # 512-query tiles per batch
VSTRIDE = NKB * 130 # vnat cols per pair: 16 blocks x [64 dims|1|64 dims|1]

F32 = mybir.dt.float32
F32R = mybir.dt.float32r
BF16 = mybir.dt.bfloat16


def build_nc():
    nc = bacc.Bacc("TRN2", target_bir_lowering=False, debug=False,
                   num_devices=N_CORES)
    xT = nc.dram_tensor("xT", [D, T], BF16, kind="ExternalInput").ap()
    wqR = nc.dram_tensor("wqR", [128, 4096], BF16, kind="ExternalInput").ap()
    wkR = nc.dram_tensor("wkR", [128, 4096], BF16, kind="ExternalInput").ap()
    wvT = nc.dram_tensor("wvT", [D, 512], BF16, kind="ExternalInput").ap()
    woT = nc.dram_tensor("woT", [512, D], BF16, kind="ExternalInput").ap()
    cpk = nc.dram_tensor("cpk", [128, 384], BF16, kind="ExternalInput").ap()
    onesb = nc.dram_tensor("onesb", [128, 128], BF16, kind="ExternalInput").ap()
    sel0 = nc.dram_tensor("sel0", [1, 128], F32R, kind="ExternalInput").ap()
    sel1 = nc.dram_tensor("sel1", [1, 128], F32R, kind="ExternalInput").ap()
    out = nc.dram_tensor("out", [T, D], F32, kind="ExternalOutput").ap()

    with tile.TileContext(nc) as tc:
        _body(tc, out, xT, wqR, wkR, wvT, woT, cpk, onesb, sel0, sel1)
    nc.compile()
    return nc


def _body(tc, out, xT, wqR, wkR, wvT, woT, cpk, onesb, sel0, sel1):
    nc = tc.nc
    from contextlib import ExitStack

    with ExitStack() as ctx:
        persist = ctx.enter_context(tc.tile_pool(name="persist", bufs=1))
        qT = persist.tile([128, PAIRS * T], BF16, tag="qT")
        kT = persist.tile([128, PAIRS * T], BF16, tag="kT")
        vnat = persist.tile([128, PAIRS * VSTRIDE], BF16, tag="vnat")
        ynorm = persist.tile([128, PAIRS * T], BF16, tag="ynorm")

        consts = ctx.enter_context(tc.tile_pool(name="consts", bufs=1))
        cpk_sb = consts.tile([128, 384], BF16, tag="cpk")
        nc.sync.dma_start(cpk_sb[:], cpk[:])
        mask2_sb = cpk_sb[:, 0:256]
        ident_sb = cpk_sb[:, 256:384]
        sel0_sb = consts.tile([1, 128], F32R, tag="sel0")
        sel1_sb = consts.tile([1, 128], F32R, tag="sel1")
        warm = consts.tile([128, 512], BF16, tag="warm")
        nc.vector.memset(warm[:], 0.0)

        wqkpool = ctx.enter_context(tc.tile_pool(name="wqk", bufs=1))
        wq_sb = wqkpool.tile([128, 4096], BF16, tag="wq")
        wk_sb = wqkpool.tile([128, 4096], BF16, tag="wk")
        wvpool = ctx.enter_context(tc.tile_pool(name="wv", bufs=1))
        wv_sb = wvpool.tile([128, 8 * 512], BF16, tag="wv")
        wopool = ctx.enter_context(tc.tile_pool(name="wo", bufs=1))
        wo_sb = []
        for p in range(PAIRS):
            wot = wopool.tile([128, 1024], BF16, tag=f"wo{p}")
            wo_sb.append(wot)

        xpool = ctx.enter_context(tc.tile_pool(name="xt", bufs=2))
        ppool = ctx.enter_context(tc.tile_pool(name="p", bufs=4))
        rpool = ctx.enter_context(tc.tile_pool(name="r", bufs=2))
        opool = ctx.enter_context(tc.tile_pool(name="osb", bufs=3))

        spool = ctx.enter_context(
            tc.tile_pool(name="s", bufs=2, space="PSUM"))
        ypool = ctx.enter_context(
            tc.tile_pool(name="y", bufs=1, space="PSUM"))
        shpool = ctx.enter_context(
            tc.tile_pool(name="sh", bufs=2, space="PSUM"))

        # ---------------- phase emitters -----------------------------
        x_sb = [None] * 8

        def load_x(half):
            for c in range(8):
                xt = xpool.tile([128, 1024], BF16, tag=f"x{c}")
                nc.sync.dma_start(
                    xt[:], xT[c * 128:(c + 1) * 128,
                              half * 1024:(half + 1) * 1024])
                x_sb[c] = xt

        def prologue_dmas():
            # warm-up burst: dummy matmuls while DMAs stream, so the PE
            # HAM un-throttles before the first real projection matmul
            wps = shpool.tile([128, 512], F32, tag="ps")
            for i in range(14):
                nc.tensor.matmul(wps[:], lhsT=warm[:, 0:128], rhs=warm[:],
                                 start=True, stop=True)
            # wq strips on sync queue, x strips on gpsimd queue: parallel
            # issue so the first q-proj matmul can start after ~0.8MB
            for c in range(8):
                nc.sync.dma_start(wq_sb[:, c * 512:(c + 1) * 512],
                                  wqR[:, c * 512:(c + 1) * 512])
                xt = xpool.tile([128, 1024], BF16, tag=f"x{c}")
                nc.gpsimd.dma_start(xt[:], xT[c * 128:(c + 1) * 128, 0:1024])
                x_sb[c] = xt
            for c in range(8):
                nc.sync.dma_start(wk_sb[:, c * 512:(c + 1) * 512],
                                  wkR[:, c * 512:(c + 1) * 512])
            ones_view = vnat[:].rearrange("r (p k m x) -> r (p k m) x",
                                          p=PAIRS, k=NKB, m=2)[:, :, 64:65]
            nc.sync.dma_start(ones_view.squeeze(), onesb[:])
            for c in range(8):
                nc.sync.dma_start(wv_sb[:, c * 512:(c + 1) * 512],
                                  wvT[c * 128:(c + 1) * 128, :])
            for p in range(PAIRS):
                nc.sync.dma_start(wo_sb[p][:], woT[p * 128:(p + 1) * 128, :])
            nc.gpsimd.dma_start(sel0_sb[:], sel0[:])
            nc.gpsimd.dma_start(sel1_sb[:], sel1[:])

        def proj_chunk(half, sub):
            # q/k for 512 tokens
            for w_sb, dst in ((wq_sb, qT), (wk_sb, kT)):
                for p in range(PAIRS):
                    ps = shpool.tile([128, 512], F32, tag="ps")
                    for c in range(8):
                        nc.tensor.matmul(
                            ps[:],
                            lhsT=(w_sb[:, c * 512 + p * 128:
                                         c * 512 + (p + 1) * 128]),
                            rhs=(x_sb[c][:, sub * 512:(sub + 1) * 512]),
                            start=(c == 0), stop=(c == 7))
                    col0 = p * T + half * 1024 + sub * 512
                    nc.vector.tensor_copy(dst[:, col0:col0 + 512], ps[:])
            # V in natural [token, dim] layout: 128-token blocks
            for tb in range(4):
                ps = shpool.tile([128, 512], F32, tag="ps")
                tok0 = sub * 512 + tb * 128
                for c in range(8):
                    nc.tensor.matmul(
                        ps[:],
                        lhsT=(x_sb[c][:, tok0:tok0 + 128]),
                        rhs=(wv_sb[:, c * 512:(c + 1) * 512]),
                        start=(c == 0), stop=(c == 7))
                kb = half * 8 + sub * 4 + tb
                srcv = ps[:].rearrange("r (p m x) -> r p m x", p=PAIRS, m=2)
                dstv = vnat[:].rearrange(
                    "r (p k m x) -> r p k m x",
                    p=PAIRS, k=NKB, m=2)[:, :, kb:kb + 1, :, 0:64]
                nc.vector.tensor_copy(dstv.squeeze(2), srcv)

        pending = [None]

        def _normalize(p, qt, y0, y1):
            den0 = rpool.tile([1, 512], F32R, tag="den0")
            den1 = rpool.tile([1, 512], F32R, tag="den1")
            nc.vector.tensor_copy(den0[:], y0[64:65, :])
            nc.vector.tensor_copy(den1[:], y1[64:65, :])
            # broadcast raw denominators to 64 partitions each (col-tiled
            # K=1 matmuls), then one fast-approx reciprocal on the full
            # [128, 512] tile (DVE cost is free-dim-bound, so this is as
            # cheap as a [1, 512] reciprocal)
            rbs = shpool.tile([128, 512], F32, tag="ps")
            nc.tensor.matmul(rbs[:], lhsT=sel0_sb[:],
                             rhs=den0[:], start=True, stop=False)
            nc.tensor.matmul(rbs[:], lhsT=sel1_sb[:],
                             rhs=den1[:], start=False, stop=True)
            rcp = rpool.tile([128, 512], F32, tag="rcp")
            nc.vector.reciprocal_approx_fast(out=rcp[:], in_=rbs[:])
            ycol = p * T + qt * 512
            nc.vector.tensor_mul(ynorm[0:64, ycol:ycol + 512],
                                 y0[0:64, :], rcp[0:64, :])
            nc.vector.tensor_mul(ynorm[64:128, ycol:ycol + 512],
                                 y1[0:64, :], rcp[64:128, :])

        def attn_qt(qt, filler=()):
            filler = list(filler)
            nkb = (qt + 1) * 4
            for p in range(PAIRS):
                if filler:
                    filler.pop(0)()
                y0 = ypool.tile([65, 512], F32, tag="y0")
                y1 = ypool.tile([65, 512], F32, tag="y1")
                for kb in range(nkb):
                    o = kb - qt * 4
                    scol = max(0, o * 128)
                    width = 512 - scol
                    qcol = p * T + qt * 512 + scol
                    kcol = p * T + kb * 128
                    vbase = p * VSTRIDE + kb * 130
                    # both heads' scores in one 2-bank PSUM tile so a
                    # single ACT instruction exponentiates both
                    s01 = spool.tile([128, 1024], F32, tag="s01")
                    nc.tensor.matmul(
                        s01[:, 0:width],
                        lhsT=(kT[0:64, kcol:kcol + 128]),
                        rhs=(qT[0:64, qcol:qcol + width]),
                        start=True, stop=(o < 0))
                    nc.tensor.matmul(
                        s01[:, 512:512 + width],
                        lhsT=(kT[64:128, kcol:kcol + 128]),
                        rhs=(qT[64:128, qcol:qcol + width]),
                        start=True, stop=(o < 0))
                    if o >= 0:
                        # causal mask: accumulate -38 into masked positions
                        # of the diagonal 128-col chunk (both heads in one
                        # N=256 matmul: fp32r needs N>=256 for full rate)
                        mview = s01[:].rearrange("r (h x) -> r h x",
                                                 h=2)[:, :, 0:128]
                        nc.tensor.matmul(
                            mview, lhsT=ident_sb,
                            rhs=mask2_sb.rearrange("r (h x) -> r h x",
                                                      h=2),
                            start=False, stop=True)
                    p01 = ppool.tile([128, 1024], BF16, tag="p01")
                    sview = s01[:].rearrange("r (h x) -> r h x",
                                             h=2)[:, :, 0:width]
                    pview = p01[:].rearrange("r (h x) -> r h x",
                                             h=2)[:, :, 0:width]
                    nc.scalar.activation(
                        pview, sview, mybir.ActivationFunctionType.Exp)
                    nc.tensor.matmul(
                        y0[:, scol:512],
                        lhsT=(vnat[:, vbase:vbase + 65]),
                        rhs=(p01[:, 0:width]),
                        start=(kb == 0), stop=(kb == nkb - 1))
                    nc.tensor.matmul(
                        y1[:, scol:512],
                        lhsT=(vnat[:, vbase + 65:vbase + 130]),
                        rhs=(p01[:, 512:512 + width]),
                        start=(kb == 0), stop=(kb == nkb - 1))
                    if kb == 1 and pending[0] is not None:
                        pending[0]()
                        pending[0] = None
                if pending[0] is not None:
                    pending[0]()
                pending[0] = (lambda p=p, qt=qt, y0=y0, y1=y1:
                              _normalize(p, qt, y0, y1))

        def oproj_tt(tt):
            osb = opool.tile([128, 1024], F32, tag="osb")
            for n in range(2):
                ps = shpool.tile([128, 512], F32, tag="ps")
                for p in range(PAIRS):
                    nc.tensor.matmul(
                        ps[:],
                        lhsT=(ynorm[:, p * T + tt * 128:
                                      p * T + tt * 128 + 128]),
                        rhs=(wo_sb[p][:, n * 512:(n + 1) * 512]),
                        start=(p == 0), stop=(p == PAIRS - 1))
                nc.vector.tensor_copy(osb[:, n * 512:(n + 1) * 512],
                                      ps[:])
            nc.gpsimd.dma_start(out[tt * 128:(tt + 1) * 128, :], osb[:])

        def oproj_filler(qt):
            # flush the last pair's normalize so ynorm for qt is complete,
            # then hand back per-token-block emitters to interleave into the
            # NEXT attention phase (fills PE during its ACT-bound stalls)
            if pending[0] is not None:
                pending[0]()
                pending[0] = None
            return [lambda tt=tt: oproj_tt(tt)
                    for tt in range(qt * 4, qt * 4 + 4)]

        def oproj_qt(qt):
            for f in oproj_filler(qt):
                f()

        # ---------------- emission order -----------------------------
        prologue_dmas()
        proj_chunk(0, 0)
        attn_qt(0)
        proj_chunk(0, 1)
        attn_qt(1, filler=oproj_filler(0))
        load_x(1)
        proj_chunk(1, 0)
        attn_qt(2, filler=oproj_filler(1))
        proj_chunk(1, 1)
        attn_qt(3, filler=oproj_filler(2))
        oproj_qt(3)


def shard_inputs(x, Wq, Wk, Wv, Wo):
    """Returns in_maps for cores 0..7 (core c: batch c//2, group c%2)."""
    import ml_dtypes
    x = np.ascontiguousarray(np.asarray(x, np.float32))
    mask1 = np.zeros((128, 128), np.float32)
    for r in range(128):
        mask1[r, :r] = -38.0  # S^T[key r, query j]: masked iff j < r
    mask = np.ascontiguousarray(np.concatenate([mask1, mask1], axis=1))
    ident = np.eye(128, dtype=np.float32)
    sel0 = np.zeros((1, 128), np.float32); sel0[0, :64] = 1.0
    sel1 = np.zeros((1, 128), np.float32); sel1[0, 64:] = 1.0
    in_maps = []
    perms = []
    for g in range(GROUPS):
        perm = np.array([(g * HPG + 2 * p + (q >= 64)) * 64 + (q % 64)
                         for p in range(PAIRS) for q in range(128)])
        perms.append(perm)
    w_cache = {}
    qscale = 1.0 / np.sqrt(DH)
    for g in range(GROUPS):
        perm = perms[g]
        wqT = (np.asarray(Wq, np.float32).T * qscale)[:, perm]
        wkT = np.asarray(Wk, np.float32).T[:, perm]
        # [r, c*512 + p*128 + o] = wT[c*128 + r, p*128 + o]
        def _re(wT):
            w4 = wT.reshape(8, 128, 4, 128)        # [c, r, p, o]
            return np.ascontiguousarray(
                w4.transpose(1, 0, 2, 3).reshape(128, 4096))
        w_cache[g] = {
            "wqR": _re(wqT).astype(ml_dtypes.bfloat16),
            "wkR": _re(wkT).astype(ml_dtypes.bfloat16),
            "wvT": np.ascontiguousarray(
                np.asarray(Wv, np.float32).T[:, perm]).astype(
                    ml_dtypes.bfloat16),
            "woT": np.ascontiguousarray(
                np.asarray(Wo, np.float32).T[perm, :]).astype(
                    ml_dtypes.bfloat16),
        }
    for c in range(N_CORES):
        b, g = c // 2, c % 2
        in_maps.append({
            "xT": np.ascontiguousarray(x[b].T).astype(ml_dtypes.bfloat16),
            "cpk": np.ascontiguousarray(
                np.concatenate([mask, ident], axis=1)).astype(
                    ml_dtypes.bfloat16),
            "onesb": np.ones((128, 128), ml_dtypes.bfloat16),
            "sel0": sel0, "sel1": sel1,
            **w_cache[g],
        })
    return in_maps


def kernel(x, Wq, Wk, Wv, Wo):
    nc = build_nc()
    in_maps = shard_inputs(x, Wq, Wk, Wv, Wo)
    res = run_bass_kernel_spmd(nc, in_maps, list(range(N_CORES)))
    out = np.empty((B, T, D), np.float32)
    for b in range(B):
        out[b] = res.results[2 * b]["out"] + res.results[2 * b + 1]["out"]
    return out
